# revision 1
# baseline (speedup 1.0000x reference)
# MoE (top-2 of 8 experts) kernel for 8 Trainium2 NeuronCores.
#
# Strategy: expert-parallel sparse routing. The reference computes every
# expert densely, but the output only depends on each token's top-2
# experts. Host computes the (tiny) gating network and per-expert token
# lists; core e runs expert e's FFN (x@W1+b1 -> LayerNorm -> erf-GELU ->
# @W2+b2) on just its routed tokens; host applies the gate weights in
# the combine. Matmuls run in float32r (TF32) at full PE rate, tokens on
# the moving (free) dimension for both matmuls so no on-device
# transposes are needed. LayerNorm reductions over H (the partition dim)
# are done with ones-vector matmuls on the PE; per-token stats are
# broadcast back across partitions with K=1 matmuls.

import tempfile

import numpy as np

import concourse.bacc as bacc
import concourse.mybir as mybir
import concourse.tile as tile
from concourse._compat import axon_active
from concourse.bass_utils import run_bass_kernel_spmd

P = 128
D, H, E, TOPK = 1024, 2048, 8, 2
DS, NJ, NK = D // P, H // P, H // P
LN_EPS = 1e-5
TT = 512          # main token tile (fp32 moving-operand max)
TT_MIN = 256      # capacity granularity; tail tiles use native-shape SBUF tiles
NJC = 8           # W2 js-slices cached in SBUF; NJ-NJC streamed per k-tile

_kernel_cache: dict[int, object] = {}


def _t_tiles(C):
    tiles, t0 = [], 0
    while t0 < C:
        tt = TT if C - t0 >= TT else TT_MIN
        tiles.append((t0, tt))
        t0 += tt
    # Put the smaller tail tile second: its DMA-heavy mm1 overlaps the
    # first 512-tile's mm2, and the final (non-overlapped) mm2 is a full
    # 512 tile with the best compute:DMA ratio.
    if len(tiles) > 1 and tiles[-1][1] != TT:
        tiles = [tiles[0], tiles[-1]] + tiles[1:-1]
    return tiles


def _build(C: int):
    f32, f32r = mybir.dt.float32, mybir.dt.float32r
    nc = bacc.Bacc("TRN2", target_bir_lowering=False, debug=False, num_devices=8)
    xT = nc.dram_tensor("xT", [P, DS, C], f32r, kind="ExternalInput").ap()
    W1 = nc.dram_tensor("W1", [NJ, P, DS, P], f32r, kind="ExternalInput").ap()
    W2 = nc.dram_tensor("W2", [P, NJC, H], f32r, kind="ExternalInput").ap()
    W2s = nc.dram_tensor("W2s", [NK, P, NJ - NJC, P], f32r, kind="ExternalInput").ap()
    b1 = nc.dram_tensor("b1", [P, NJ], f32, kind="ExternalInput").ap()
    lg = nc.dram_tensor("lg", [P, NJ], f32, kind="ExternalInput").ap()
    lb = nc.dram_tensor("lb", [P, NJ], f32, kind="ExternalInput").ap()
    b2 = nc.dram_tensor("b2", [P, NK], f32, kind="ExternalInput").ap()
    outT = nc.dram_tensor("outT", [NK, P, C], f32, kind="ExternalOutput").ap()

    Gelu = mybir.ActivationFunctionType.Gelu
    Ident = mybir.ActivationFunctionType.Identity
    Sqrt = mybir.ActivationFunctionType.Sqrt

    with tile.TileContext(nc) as tc:
        with (
            tc.tile_pool(name="const", bufs=1) as constp,
            tc.tile_pool(name="w2p", bufs=1) as w2p,
            tc.tile_pool(name="w1p", bufs=2) as w1p,
            tc.tile_pool(name="w2sp", bufs=2) as w2sp,
            tc.tile_pool(name="xp", bufs=1) as xp,
            tc.tile_pool(name="hp", bufs=2) as hp,
            tc.tile_pool(name="sqp", bufs=2) as sqp,
            tc.tile_pool(name="op", bufs=2) as op,
            tc.tile_pool(name="statp", bufs=1) as statp,
            tc.tile_pool(name="ps_mm", bufs=3, space="PSUM") as ps_mm,
            tc.tile_pool(name="ps_acc", bufs=1, space="PSUM") as ps_acc,
            tc.tile_pool(name="ps_bc", bufs=1, space="PSUM") as ps_bc,
        ):
            b1s = constp.tile([P, NJ], f32)
            nc.sync.dma_start(b1s[:], b1[:])
            lgs = constp.tile([P, NJ], f32)
            nc.sync.dma_start(lgs[:], lg[:])
            lbs = constp.tile([P, NJ], f32)
            nc.sync.dma_start(lbs[:], lb[:])
            b2s = constp.tile([P, NK], f32)
            nc.sync.dma_start(b2s[:], b2[:])
            ones_f = constp.tile([P, 1], f32)
            nc.any.memset(ones_f[:], 1.0)
            ones_c = constp.tile([P, 1], f32r)  # lhsT for partition-sum matmuls
            nc.vector.tensor_copy(ones_c[:], ones_f[:])
            oner_f = constp.tile([1, P], f32)
            nc.any.memset(oner_f[:], 1.0)
            oner_c = constp.tile([1, P], f32r)  # lhsT for partition-broadcast matmuls
            nc.vector.tensor_copy(oner_c[:], oner_f[:])
            eps_t = constp.tile([1, 1], f32)
            nc.any.memset(eps_t[:], LN_EPS)

            # Cache js slices [0, NJC) of W2 in SBUF (f32r); stream the rest
            # per k-tile (they don't fit alongside both t-tile shapes). The
            # DMAs are emitted after the first tile's xt/W1 loads (below) so
            # they don't delay the first matmuls; first use is ~90us in.
            w2sb = w2p.tile([P, NJC, H], f32r)

            def emit_w2c_chunk(c):
                if c < NJC:
                    nc.sync.dma_start(
                        w2sb[:, c : c + 1, :], W2[:, c : c + 1, :]
                    )

            def emit_mm2(h, t0, tt):
                for k in range(NK):
                    w2t = w2sp.tile([P, NJ - NJC, P], f32r, tag="w2s", name="w2t")
                    nc.sync.dma_start(w2t[:], W2s[k])
                    pm = ps_mm.tile([P, TT], f32, tag="mm", name="mm")[:, :tt]
                    for js in range(NJ):
                        nc.tensor.matmul(
                            pm[:],
                            w2sb[:, js, k * P : (k + 1) * P]
                            if js < NJC
                            else w2t[:, js - NJC, :],
                            h[:, js, :],
                            start=(js == 0),
                            stop=(js == NJ - 1),
                        )
                    ot = op.tile(
                        [P, tt], f32, tag=f"out{tt}", name="out",
                        bufs=(2 if tt == TT else 1),
                    )
                    nc.scalar.activation(ot[:], pm[:], Ident, bias=b2s[:, k : k + 1])
                    nc.sync.dma_start(outT[k, :, t0 : t0 + tt], ot[:])

            # Software-pipelined emission: tile i's mm2 is emitted after tile
            # i+1's mm1, so the PE runs mm2(i) while ACT/DVE do tile i+1's
            # LayerNorm stats, and runs mm1(i+1) while normalize/gelu(i+1)
            # complete. h is double-buffered to allow it.
            prev = None
            first = True
            for t0, tt in _t_tiles(C):
                xt = xp.tile([P, DS, tt], f32r, tag=f"xt{tt}", name="xt")
                nc.sync.dma_start(xt[:], xT[:, :, t0 : t0 + tt])
                h = hp.tile(
                    [P, NJ, tt], f32r, tag=f"h{tt}", name="h",
                    bufs=(2 if tt == TT else 1),
                )
                s_ps = ps_acc.tile([1, TT], f32, tag="sacc", name="sacc")[:, :tt]
                q_ps = ps_acc.tile([1, TT], f32, tag="qacc", name="qacc")[:, :tt]

                # ---- mm1; S/Q ones-matmuls deferred by one j so the PE
                # never waits on the ACT evict / DVE square chain ----
                def emit_snq(j, sq):
                    nc.tensor.matmul(
                        s_ps[:], ones_c[:], h[:, j, :],
                        start=(j == 0), stop=(j == NJ - 1),
                    )
                    nc.tensor.matmul(
                        q_ps[:], ones_c[:], sq[:],
                        start=(j == 0), stop=(j == NJ - 1),
                    )

                pend = None
                for j in range(NJ):
                    w1t = w1p.tile([P, DS, P], f32r, tag="w1")
                    nc.sync.dma_start(w1t[:], W1[j])
                    if first:
                        emit_w2c_chunk(j)
                        emit_w2c_chunk(j + NJ)
                    pm = ps_mm.tile([P, TT], f32, tag="mm", name="mm")[:, :tt]
                    for ds in range(DS):
                        nc.tensor.matmul(
                            pm[:],
                            w1t[:, ds, :],
                            xt[:, ds, :],
                            start=(ds == 0),
                            stop=(ds == DS - 1),
                        )
                    # evict psum -> h (f32r) with per-partition bias b1[j]
                    nc.scalar.activation(
                        h[:, j, :], pm[:], Ident, bias=b1s[:, j : j + 1]
                    )
                    sq = sqp.tile([P, tt], f32r, tag=f"sq{tt}", name="sq")
                    nc.vector.tensor_mul(sq[:], h[:, j, :], h[:, j, :])
                    if pend is not None:
                        emit_snq(*pend)
                    pend = (j, sq)
                emit_snq(*pend)
                first = False

                # previous tile's mm2 fills the PE while this tile's stats run
                if prev is not None:
                    emit_mm2(*prev)

                # ---- LN stats -> per-token scale A=rstd and offset B=mu*rstd ----
                mu = statp.tile([1, TT], f32, tag="mu", name="mu")[:, :tt]
                nc.vector.tensor_scalar_mul(mu[:], s_ps[:], 1.0 / H)
                tmp = statp.tile([1, TT], f32, tag="tmp", name="tmp")[:, :tt]
                nc.vector.tensor_scalar_mul(tmp[:], q_ps[:], 1.0 / H)
                tmp2 = statp.tile([1, TT], f32, tag="tmp2", name="tmp2")[:, :tt]
                nc.vector.tensor_mul(tmp2[:], mu[:], mu[:])
                nc.vector.tensor_sub(tmp[:], tmp[:], tmp2[:])          # var
                nc.scalar.activation(tmp2[:], tmp[:], Sqrt, bias=eps_t[:])  # std
                nc.vector.reciprocal(tmp[:], tmp2[:])                  # rstd
                a_row = statp.tile([1, TT], f32r, tag="a_row", name="a_row")[:, :tt]
                nc.vector.tensor_copy(a_row[:], tmp[:])
                b_row = statp.tile([1, TT], f32r, tag="b_row", name="b_row")[:, :tt]
                nc.vector.tensor_mul(b_row[:], mu[:], tmp[:])
                # broadcast across partitions via K=1 matmuls
                a_bc = ps_bc.tile([P, TT], f32, tag="a_bc", name="a_bc")[:, :tt]
                nc.tensor.matmul(a_bc[:], oner_c[:], a_row[:], start=True, stop=True)
                b_bc = ps_bc.tile([P, TT], f32, tag="b_bc", name="b_bc")[:, :tt]
                nc.tensor.matmul(b_bc[:], oner_c[:], b_row[:], start=True, stop=True)

                # ---- normalize + affine + GELU (in place on h) ----
                for j in range(NJ):
                    hj = h[:, j, :]
                    nc.vector.tensor_mul(hj, hj, a_bc[:])
                    nc.vector.tensor_sub(hj, hj, b_bc[:])
                    nc.scalar.activation(
                        hj, hj, Gelu, bias=lbs[:, j : j + 1], scale=lgs[:, j : j + 1]
                    )
                prev = (h, t0, tt)

            emit_mm2(*prev)

    nc.compile()
    return nc


def _route(x64, Wg64, bg64):
    """Host gating: returns per-token top-2 expert ids and renormalized weights."""
    logits = x64 @ Wg64 + bg64                      # [N, E] fp64
    order = np.argsort(-logits, axis=1, kind="stable")[:, :TOPK]
    l0 = np.take_along_axis(logits, order, axis=1)  # [N, 2] descending
    # pair softmax == softmax-then-renormalize over the top-2
    w0 = 1.0 / (1.0 + np.exp(l0[:, 1] - l0[:, 0]))
    w = np.stack([w0, 1.0 - w0], axis=1)
    return order, w


def kernel(x, W1, b1, ln_g, ln_b, W2, b2, Wg, bg):
    x = np.ascontiguousarray(np.asarray(x, dtype=np.float32))
    W1 = np.asarray(W1, dtype=np.float32)
    b1 = np.asarray(b1, dtype=np.float32)
    ln_g = np.asarray(ln_g, dtype=np.float32)
    ln_b = np.asarray(ln_b, dtype=np.float32)
    W2 = np.asarray(W2, dtype=np.float32)
    b2 = np.asarray(b2, dtype=np.float32)
    Wg = np.asarray(Wg, dtype=np.float32)
    bg = np.asarray(bg, dtype=np.float32)
    N = x.shape[0]

    order, w = _route(x.astype(np.float64), Wg.astype(np.float64), bg.astype(np.float64))

    # Per-expert token lists, padded to a common capacity C (multiple of TT_MIN).
    tok_idx, tok_w = [], []
    for e in range(E):
        sel = np.nonzero((order[:, 0] == e) | (order[:, 1] == e))[0]
        we = np.where(order[sel, 0] == e, w[sel, 0], w[sel, 1]).astype(np.float32)
        tok_idx.append(sel)
        tok_w.append(we)
    C = max(TT_MIN, int(-(-max(len(s) for s in tok_idx) // TT_MIN)) * TT_MIN)

    if C not in _kernel_cache:
        _kernel_cache[C] = _build(C)
    nc = _kernel_cache[C]

    in_maps = []
    for e in range(E):
        idx = np.zeros(C, dtype=np.int64)
        idx[: len(tok_idx[e])] = tok_idx[e]
        xg = x[idx]                                   # [C, D]
        xT_dev = np.ascontiguousarray(xg.reshape(C, DS, P).transpose(2, 1, 0))
        W1_dev = np.ascontiguousarray(
            W1[e].reshape(DS, P, NJ, P).transpose(2, 1, 0, 3)
        )
        w2r = W2[e].reshape(NJ, P, H)
        W2_dev = np.ascontiguousarray(w2r[:NJC].transpose(1, 0, 2))
        W2s_dev = np.ascontiguousarray(
            W2[e][NJC * P :, :].reshape(NJ - NJC, P, NK, P).transpose(2, 1, 0, 3)
        )
        in_maps.append(
            {
                "xT": xT_dev,
                "W1": W1_dev,
                "W2": W2_dev,
                "W2s": W2s_dev,
                "b1": np.ascontiguousarray(b1[e].reshape(NJ, P).T),
                "lg": np.ascontiguousarray(ln_g[e].reshape(NJ, P).T),
                "lb": np.ascontiguousarray(ln_b[e].reshape(NJ, P).T),
                "b2": np.ascontiguousarray(b2[e].reshape(NK, P).T),
            }
        )

    results = _run(C, nc, in_maps)

    y = np.zeros((N, H), dtype=np.float32)
    for e in range(E):
        cnt = len(tok_idx[e])
        eoT = results[e]["outT"].reshape(H, C)
        y[tok_idx[e]] += tok_w[e][:, None] * eoT[:, :cnt].T
    return y


_neff_cache: dict[int, str] = {}


def _run(C, nc, in_maps):
    if axon_active():
        # PJRT path; NEFF compile is cached by libneuronxla.
        return run_bass_kernel_spmd(nc, in_maps, core_ids=list(range(E))).results
    # Native path: compile once per capacity, then execute the cached NEFF.
    from concourse.bass_utils import compile_bass_kernel, run_neff

    if C not in _neff_cache:
        _neff_cache[C] = compile_bass_kernel(nc, tempfile.mkdtemp())
    out_maps = [
        {"outT": np.zeros((NK, P, C), dtype=np.float32)} for _ in range(E)
    ]
    in_maps = [m.copy() for m in in_maps]
    if nc.partition_id_tensor:
        for core_id, m in enumerate(in_maps):
            m[nc.partition_id_tensor.name] = np.array([[core_id]], dtype=np.uint32)
    return run_neff(
        _neff_cache[C],
        in_maps,
        out_maps,
        core_ids=list(range(E)),
        has_collectives=False,
    )



# revision 5
# speedup vs baseline: 1.3883x; 1.3883x over previous
# MoE (top-2 of 8 experts) kernel for 8 Trainium2 NeuronCores.
#
# Strategy: expert-parallel sparse routing with fp8 DoubleRow matmuls.
# Host computes the gating network and per-expert token lists; core e runs
# expert e's FFN (x@W1+b1 -> LayerNorm -> erf-GELU -> @W2+b2) on its routed
# tokens. Both matmuls run as fp8(e4m3) DoubleRow (2 k-planes per
# instruction, 0.5 cyc/row = 4x the f32r MAC rate) with hi/lo error
# compensation: A@B ~= Ah@Bh + Al@Bh + Ah@Bl where Ah=fp8(A), Al=fp8(A-Ah).
# All weights live in SBUF (12MB fp8), loaded once. LayerNorm sums use a
# bf16 ones-matmul (S) and a paired-fp8 DoubleRow ones-matmul on h^2 (Q);
# per-token stats broadcast across partitions with K=1 f32r matmuls. b2 is
# folded into the mm2 PSUM chain via a constant fp8 matmul so the mm2 evict
# is a single DVE tensor_scalar_mul (keeps ACT free for the GELUs).

import tempfile

import ml_dtypes
import numpy as np

import concourse.bacc as bacc
import concourse.mybir as mybir
import concourse.tile as tile
from concourse._compat import axon_active
from concourse.bass_utils import run_bass_kernel_spmd

P = 128
D, H, E, TOPK = 1024, 2048, 8, 2
DP, KP, NJ, NK = D // 256, H // 256, H // P, H // P  # 4, 8, 16, 16
LN_EPS = 1e-5
TT = 512          # main token tile
GRAN = 128        # capacity granularity
SX, SW1, SW2 = 16.0, 256.0, 256.0   # fp8 pre-quantization scales
F8 = ml_dtypes.float8_e4m3
BF = ml_dtypes.bfloat16

_kernel_cache: dict[int, object] = {}


def _t_tiles(C):
    tiles, t0 = [], 0
    while t0 < C:
        tt = TT if C - t0 >= TT else C - t0
        tiles.append((t0, tt))
        t0 += tt
    # Put the small tail tile second so the final (least overlapped) mm2 is
    # a full 512 tile.
    if len(tiles) > 1 and tiles[-1][1] != TT:
        tiles = [tiles[0], tiles[-1]] + tiles[1:-1]
    return tiles


def _build(C: int):
    f32, f32r, bf16, f8 = (
        mybir.dt.float32, mybir.dt.float32r, mybir.dt.bfloat16, mybir.dt.float8e4
    )
    DR = mybir.MatmulPerfMode.DoubleRow
    nc = bacc.Bacc("TRN2", target_bir_lowering=False, debug=False, num_devices=8)
    XH = nc.dram_tensor("XH", [P, DP, 2, C], f8, kind="ExternalInput").ap()
    XL = nc.dram_tensor("XL", [P, DP, 2, C], f8, kind="ExternalInput").ap()
    W1H = nc.dram_tensor("W1H", [P, DP, 2, NJ, P], f8, kind="ExternalInput").ap()
    W1L = nc.dram_tensor("W1L", [P, DP, 2, NJ, P], f8, kind="ExternalInput").ap()
    W2H = nc.dram_tensor("W2H", [P, KP, 2, NK, P], f8, kind="ExternalInput").ap()
    W2L = nc.dram_tensor("W2L", [P, KP, 2, NK, P], f8, kind="ExternalInput").ap()
    B2B = nc.dram_tensor("B2B", [P, NK, 2, P], f8, kind="ExternalInput").ap()
    b1 = nc.dram_tensor("b1", [P, NJ], f32, kind="ExternalInput").ap()
    lg = nc.dram_tensor("lg", [P, NJ], f32, kind="ExternalInput").ap()
    lb = nc.dram_tensor("lb", [P, NJ], f32, kind="ExternalInput").ap()
    outT = nc.dram_tensor("outT", [NK, P, C], bf16, kind="ExternalOutput").ap()

    Gelu = mybir.ActivationFunctionType.Gelu
    Ident = mybir.ActivationFunctionType.Identity
    Sqrt = mybir.ActivationFunctionType.Sqrt

    with tile.TileContext(nc) as tc:
        with (
            tc.tile_pool(name="const", bufs=1) as constp,
            tc.tile_pool(name="wp", bufs=1) as wp,
            tc.tile_pool(name="xp", bufs=1) as xp,
            tc.tile_pool(name="hp", bufs=2) as hp,
            tc.tile_pool(name="hxp", bufs=2) as hxp,
            tc.tile_pool(name="sqp", bufs=2) as sqp,
            tc.tile_pool(name="op", bufs=2) as op,
            tc.tile_pool(name="statp", bufs=1) as statp,
            tc.tile_pool(name="ps_mm", bufs=3, space="PSUM") as ps_mm,
            tc.tile_pool(name="ps_acc", bufs=1, space="PSUM") as ps_acc,
            tc.tile_pool(name="ps_bc", bufs=1, space="PSUM") as ps_bc,
        ):
            b1s = constp.tile([P, NJ], f32)
            nc.sync.dma_start(b1s[:], b1[:])
            lgs = constp.tile([P, NJ], f32)
            nc.sync.dma_start(lgs[:], lg[:])
            lbs = constp.tile([P, NJ], f32)
            nc.sync.dma_start(lbs[:], lb[:])
            ones_f = constp.tile([P, 32], f32)
            nc.any.memset(ones_f[:], 1.0)
            ones_bf = constp.tile([P, 32], bf16)  # lhsT for S partition-sums
            nc.vector.tensor_copy(ones_bf[:], ones_f[:])
            ones_q = constp.tile([P, 2, 32], f8)  # lhsT for Q paired DR sums
            nc.any.memset(ones_q[:], 1.0)
            ones_b2 = constp.tile([P, 2, TT], f8)  # rhs for the b2-fold matmul
            nc.any.memset(ones_b2[:], 1.0 / 16.0)
            oner_f = constp.tile([1, P], f32)
            nc.any.memset(oner_f[:], 1.0)
            oner_c = constp.tile([1, P], f32r)    # lhsT for partition-broadcasts
            nc.vector.tensor_copy(oner_c[:], oner_f[:])
            eps_t = constp.tile([1, 1], f32)
            nc.any.memset(eps_t[:], LN_EPS)

            # fp8 weights, SBUF-resident for the whole kernel. DMAs for the
            # first-tile W1 slices are emitted inside the first j-loop so
            # compute starts immediately; W2/B2B stream in behind them.
            w1h = wp.tile([P, DP, 2, NJ, P], f8)
            w1l = wp.tile([P, DP, 2, NJ, P], f8)
            w2h = wp.tile([P, KP, 2, NK, P], f8)
            w2l = wp.tile([P, KP, 2, NK, P], f8)
            b2b = wp.tile([P, NK, 2, P], f8)

            def emit_w_chunk(c):
                # one chunk per j-step of the first tile: W1 j-slices first
                # (needed now), then W2/B2B behind them
                if c < NJ:
                    nc.sync.dma_start(w1h[:, :, :, c, :], W1H[:, :, :, c, :])
                    nc.sync.dma_start(w1l[:, :, :, c, :], W1L[:, :, :, c, :])
                elif c < 2 * NJ:
                    k = c - NJ
                    nc.sync.dma_start(w2h[:, :, :, k, :], W2H[:, :, :, k, :])
                    nc.sync.dma_start(w2l[:, :, :, k, :], W2L[:, :, :, k, :])
                    if k == 0:
                        nc.sync.dma_start(b2b[:], B2B[:])

            def emit_mm2(h_hi, h_lo, t0, tt):
                # 3-pass compensated fp8 mm2 + b2-fold, evict on DVE
                for k in range(NK):
                    pm = ps_mm.tile([P, TT], f32, tag="mm", name="mm2")[:, :tt]
                    for pi, (wt, ht) in enumerate(
                        ((w2h, h_hi), (w2h, h_lo), (w2l, h_hi))
                    ):
                        for kp in range(KP):
                            nc.tensor.matmul(
                                pm[:],
                                wt[:, kp, :, k, :],
                                ht[:, kp, :, :tt],
                                start=(pi == 0 and kp == 0),
                                stop=False,
                                perf_mode=DR,
                            )
                    nc.tensor.matmul(
                        pm[:], b2b[:, k, :, :], ones_b2[:, :, :tt],
                        start=False, stop=True, perf_mode=DR,
                    )
                    ot = op.tile(
                        [P, tt], bf16, tag=f"out{tt}", name="out",
                        bufs=(2 if tt == TT else 1),
                    )
                    nc.vector.tensor_scalar_mul(ot[:], pm[:], 1.0 / SW2)
                    nc.sync.dma_start(outT[k, :, t0 : t0 + tt], ot[:])
                    yield k

            def emit_norm_gelu(h, h_hi, h_lo, j, tt):
                # normalize (DVE bf16) + GELU twice (ACT: fp8 h_hi, bf16 in
                # place) + h_lo residual (DVE)
                jp, pl = j // 2, j % 2
                hj = h[:, j, :tt]
                nc.vector.tensor_mul(hj, hj, a_sb[:, :tt])
                nc.vector.tensor_sub(hj, hj, b_sb[:, :tt])
                nc.scalar.activation(
                    h_hi[:, jp, pl, :tt], hj, Gelu,
                    bias=lbs[:, j : j + 1], scale=lgs[:, j : j + 1],
                )
                nc.scalar.activation(
                    hj, hj, Gelu, bias=lbs[:, j : j + 1], scale=lgs[:, j : j + 1]
                )
                nc.vector.tensor_sub(h_lo[:, jp, pl, :tt], hj, h_hi[:, jp, pl, :tt])

            prev = None
            first = True
            emit_w_chunk(0)
            for t0, tt in _t_tiles(C):
                xh = xp.tile([P, DP, 2, TT], f8, tag="xh", name="xh")
                xl = xp.tile([P, DP, 2, TT], f8, tag="xl", name="xl")
                nc.sync.dma_start(xh[:, :, :, :tt], XH[:, :, :, t0 : t0 + tt])
                nc.sync.dma_start(xl[:, :, :, :tt], XL[:, :, :, t0 : t0 + tt])
                h = hp.tile(
                    [P, NJ, tt], bf16, tag=f"h{tt}", name="h",
                    bufs=(2 if tt == TT else 1),
                )
                h_hi = hxp.tile(
                    [P, KP, 2, tt], f8, tag=f"hh{tt}", name="h_hi",
                    bufs=(2 if tt == TT else 1),
                )
                h_lo = hxp.tile(
                    [P, KP, 2, tt], f8, tag=f"hl{tt}", name="h_lo",
                    bufs=(2 if tt == TT else 1),
                )
                s_ps = ps_acc.tile([32, TT], f32, tag="sacc", name="sacc")[:, :tt]
                q_ps = ps_acc.tile([32, TT], f32, tag="qacc", name="qacc")[:, :tt]

                # ---- mm1 (3-pass fp8 DR); S/Q ones-matmuls deferred so the
                # PE never waits on the ACT evict / DVE square chain ----
                pend_s = None
                pend_q = None
                sq = None
                for j in range(NJ):
                    if first:
                        emit_w_chunk(2 * j + 1)
                        emit_w_chunk(2 * j + 2)
                    pm = ps_mm.tile([P, TT], f32, tag="mm", name="mm1")[:, :tt]
                    for pi, (wt, xt) in enumerate(((w1h, xh), (w1h, xl), (w1l, xh))):
                        for dp in range(DP):
                            nc.tensor.matmul(
                                pm[:],
                                wt[:, dp, :, j, :],
                                xt[:, dp, :, :tt],
                                start=(pi == 0 and dp == 0),
                                stop=(pi == 2 and dp == DP - 1),
                                perf_mode=DR,
                            )
                    nc.scalar.activation(
                        h[:, j, :], pm[:], Ident,
                        bias=b1s[:, j : j + 1], scale=1.0 / (SX * SW1),
                    )
                    if j % 2 == 0:
                        sq = sqp.tile([P, 2, TT], f8, tag="sq", name="sq")
                    nc.vector.tensor_mul(sq[:, j % 2, :tt], h[:, j, :], h[:, j, :])
                    if pend_s is not None:
                        jj = pend_s
                        nc.tensor.matmul(
                            s_ps[:], ones_bf[:], h[:, jj, :],
                            start=(jj == 0), stop=(jj == NJ - 1),
                        )
                    pend_s = j
                    if j % 2 == 1:
                        if pend_q is not None:
                            jp, sqt = pend_q
                            nc.tensor.matmul(
                                q_ps[:], ones_q[:], sqt[:, :, :tt],
                                start=(jp == 0), stop=(jp == NJ // 2 - 1),
                                perf_mode=DR,
                            )
                        pend_q = (j // 2, sq)
                jj = pend_s
                nc.tensor.matmul(
                    s_ps[:], ones_bf[:], h[:, jj, :],
                    start=(jj == 0), stop=(jj == NJ - 1),
                )
                jp, sqt = pend_q
                nc.tensor.matmul(
                    q_ps[:], ones_q[:], sqt[:, :, :tt],
                    start=(jp == 0), stop=(jp == NJ // 2 - 1),
                    perf_mode=DR,
                )
                first = False

                # ---- LN stats -> A=rstd, B=mu*rstd rows, broadcast, bf16 ----
                mu = statp.tile([1, TT], f32, tag="mu", name="mu")[:, :tt]
                nc.vector.tensor_scalar_mul(mu[:], s_ps[0:1, :], 1.0 / H)
                tmp = statp.tile([1, TT], f32, tag="tmp", name="tmp")[:, :tt]
                nc.vector.tensor_scalar_mul(tmp[:], q_ps[0:1, :], 1.0 / H)
                tmp2 = statp.tile([1, TT], f32, tag="tmp2", name="tmp2")[:, :tt]
                nc.vector.tensor_mul(tmp2[:], mu[:], mu[:])
                nc.vector.tensor_sub(tmp[:], tmp[:], tmp2[:])          # var
                nc.scalar.activation(tmp2[:], tmp[:], Sqrt, bias=eps_t[:])  # std
                nc.vector.reciprocal(tmp[:], tmp2[:])                  # rstd
                a_row = statp.tile([1, TT], f32r, tag="a_row", name="a_row")[:, :tt]
                nc.vector.tensor_copy(a_row[:], tmp[:])
                b_row = statp.tile([1, TT], f32r, tag="b_row", name="b_row")[:, :tt]
                nc.vector.tensor_mul(b_row[:], mu[:], tmp[:])
                a_bc = ps_bc.tile([P, TT], f32, tag="a_bc", name="a_bc")[:, :tt]
                nc.tensor.matmul(a_bc[:], oner_c[:], a_row[:], start=True, stop=True)
                b_bc = ps_bc.tile([P, TT], f32, tag="b_bc", name="b_bc")[:, :tt]
                nc.tensor.matmul(b_bc[:], oner_c[:], b_row[:], start=True, stop=True)
                a_sb = statp.tile([P, TT], bf16, tag="a_sb", name="a_sb", bufs=2)
                nc.vector.tensor_copy(a_sb[:, :tt], a_bc[:])
                b_sb = statp.tile([P, TT], bf16, tag="b_sb", name="b_sb", bufs=2)
                nc.vector.tensor_copy(b_sb[:, :tt], b_bc[:])

                # ---- previous tile's mm2 on the PE, interleaved with this
                # tile's normalize+GELU+hi/lo split on DVE/ACT ----
                if prev is not None:
                    ph_hi, ph_lo, pt0, ptt = prev
                    for k in emit_mm2(ph_hi, ph_lo, pt0, ptt):
                        emit_norm_gelu(h, h_hi, h_lo, k, tt)
                else:
                    for j in range(NJ):
                        emit_norm_gelu(h, h_hi, h_lo, j, tt)
                prev = (h_hi, h_lo, t0, tt)

            for _ in emit_mm2(*prev):
                pass

    nc.compile()
    return nc


def _route(x64, Wg64, bg64):
    """Host gating: per-token top-2 expert ids and renormalized weights."""
    logits = x64 @ Wg64 + bg64                      # [N, E] fp64
    order = np.argsort(-logits, axis=1, kind="stable")[:, :TOPK]
    l0 = np.take_along_axis(logits, order, axis=1)  # [N, 2] descending
    w0 = 1.0 / (1.0 + np.exp(l0[:, 1] - l0[:, 0]))
    w = np.stack([w0, 1.0 - w0], axis=1)
    return order, w


def _split8(a):
    hi = a.astype(F8)
    lo = (a - hi.astype(np.float32)).astype(F8)
    return hi, lo


def kernel(x, W1, b1, ln_g, ln_b, W2, b2, Wg, bg):
    x = np.ascontiguousarray(np.asarray(x, dtype=np.float32))
    W1 = np.asarray(W1, dtype=np.float32)
    b1 = np.asarray(b1, dtype=np.float32)
    ln_g = np.asarray(ln_g, dtype=np.float32)
    ln_b = np.asarray(ln_b, dtype=np.float32)
    W2 = np.asarray(W2, dtype=np.float32)
    b2 = np.asarray(b2, dtype=np.float32)
    Wg = np.asarray(Wg, dtype=np.float32)
    bg = np.asarray(bg, dtype=np.float32)
    N = x.shape[0]

    order, w = _route(x.astype(np.float64), Wg.astype(np.float64), bg.astype(np.float64))

    tok_idx, tok_w = [], []
    for e in range(E):
        sel = np.nonzero((order[:, 0] == e) | (order[:, 1] == e))[0]
        we = np.where(order[sel, 0] == e, w[sel, 0], w[sel, 1]).astype(np.float32)
        tok_idx.append(sel)
        tok_w.append(we)
    C = max(GRAN, int(-(-max(len(s) for s in tok_idx) // GRAN)) * GRAN)

    if C not in _kernel_cache:
        _kernel_cache[C] = _build(C)
    nc = _kernel_cache[C]

    in_maps = []
    for e in range(E):
        idx = np.zeros(C, dtype=np.int64)
        idx[: len(tok_idx[e])] = tok_idx[e]
        xg = x[idx] * SX                              # [C, D]
        xh, xl = _split8(xg)
        # [C, D] -> [P, DP, 2, C]
        xh_d = np.ascontiguousarray(xh.reshape(C, DP, 2, P).transpose(3, 1, 2, 0))
        xl_d = np.ascontiguousarray(xl.reshape(C, DP, 2, P).transpose(3, 1, 2, 0))
        w1h, w1l = _split8(W1[e] * SW1)               # [D, H]
        w1h_d = np.ascontiguousarray(
            w1h.reshape(DP, 2, P, NJ, P).transpose(2, 0, 1, 3, 4)
        )
        w1l_d = np.ascontiguousarray(
            w1l.reshape(DP, 2, P, NJ, P).transpose(2, 0, 1, 3, 4)
        )
        w2h, w2l = _split8(W2[e] * SW2)               # [H, H]
        w2h_d = np.ascontiguousarray(
            w2h.reshape(KP, 2, P, NK, P).transpose(2, 0, 1, 3, 4)
        )
        w2l_d = np.ascontiguousarray(
            w2l.reshape(KP, 2, P, NK, P).transpose(2, 0, 1, 3, 4)
        )
        # b2 fold: contribution = sum_{p,pl} B2B[p,k,pl,m] * (1/16)
        #        = 128*(q0+q1)/16 = 8*(32*b2) = SW2*b2
        q0, q1 = _split8(32.0 * b2[e])                # [H]
        b2b = np.stack([q0, q1], axis=0).reshape(2, NK, P).transpose(1, 0, 2)
        b2b_d = np.ascontiguousarray(
            np.broadcast_to(b2b[None], (P, NK, 2, P)).astype(F8)
        )
        in_maps.append(
            {
                "XH": xh_d,
                "XL": xl_d,
                "W1H": w1h_d,
                "W1L": w1l_d,
                "W2H": w2h_d,
                "W2L": w2l_d,
                "B2B": b2b_d,
                "b1": np.ascontiguousarray(b1[e].reshape(NJ, P).T),
                "lg": np.ascontiguousarray(ln_g[e].reshape(NJ, P).T),
                "lb": np.ascontiguousarray(ln_b[e].reshape(NJ, P).T),
            }
        )

    results = _run(C, nc, in_maps)

    y = np.zeros((N, H), dtype=np.float32)
    for e in range(E):
        cnt = len(tok_idx[e])
        eoT = results[e]["outT"].reshape(H, C).astype(np.float32)
        y[tok_idx[e]] += tok_w[e][:, None] * eoT[:, :cnt].T
    return y


_neff_cache: dict[int, str] = {}


def _run(C, nc, in_maps):
    if axon_active():
        # PJRT path; NEFF compile is cached by libneuronxla.
        return run_bass_kernel_spmd(nc, in_maps, core_ids=list(range(E))).results
    # Native path: compile once per capacity, then execute the cached NEFF.
    from concourse.bass_utils import compile_bass_kernel, run_neff

    if C not in _neff_cache:
        _neff_cache[C] = compile_bass_kernel(nc, tempfile.mkdtemp())
    out_maps = [{"outT": np.zeros((NK, P, C), dtype=BF)} for _ in range(E)]
    in_maps = [m.copy() for m in in_maps]
    if nc.partition_id_tensor:
        for core_id, m in enumerate(in_maps):
            m[nc.partition_id_tensor.name] = np.array([[core_id]], dtype=np.uint32)
    return run_neff(
        _neff_cache[C],
        in_maps,
        out_maps,
        core_ids=list(range(E)),
        has_collectives=False,
    )


# revision 7
# speedup vs baseline: 1.4572x; 1.0496x over previous
# MoE (top-2 of 8 experts) kernel for 8 Trainium2 NeuronCores.
#
# Strategy: expert-parallel sparse routing with fp8 DoubleRow matmuls.
# Host computes the gating network and per-expert token lists; core e runs
# expert e's FFN (x@W1+b1 -> LayerNorm -> erf-GELU -> @W2+b2) on its routed
# tokens. Both matmuls run as fp8(e4m3) DoubleRow (2 k-planes per
# instruction, 0.5 cyc/row = 4x the f32r MAC rate) with hi/lo error
# compensation: A@B ~= Ah@Bh + Al@Bh + Ah@Bl where Ah=fp8(A), Al=fp8(A-Ah).
# All weights live in SBUF (12MB fp8), loaded once. LayerNorm sums use a
# bf16 ones-matmul (S) and a paired-fp8 DoubleRow ones-matmul on h^2 (Q);
# per-token stats broadcast across partitions with K=1 f32r matmuls. b2 is
# folded into the mm2 PSUM chain via a constant fp8 matmul so the mm2 evict
# is a single DVE tensor_scalar_mul (keeps ACT free for the GELUs).

import tempfile

import ml_dtypes
import numpy as np

import concourse.bacc as bacc
import concourse.mybir as mybir
import concourse.tile as tile
from concourse._compat import axon_active
from concourse.bass_utils import run_bass_kernel_spmd

P = 128
D, H, E, TOPK = 1024, 2048, 8, 2
DP, KP, NJ, NK = D // 256, H // 256, H // P, H // P  # 4, 8, 16, 16
LN_EPS = 1e-5
TT = 512          # main token tile
GRAN = 128        # capacity granularity
SX, SW1, SW2 = 16.0, 256.0, 256.0   # fp8 pre-quantization scales
F8 = ml_dtypes.float8_e4m3
BF = ml_dtypes.bfloat16

_kernel_cache: dict[int, object] = {}


def _t_tiles(C):
    tiles, t0 = [], 0
    while t0 < C:
        tt = TT if C - t0 >= TT else C - t0
        tiles.append((t0, tt))
        t0 += tt
    # Tail tile stays last: its cheap mm2 is the only un-overlapped one,
    # and full-size norm/GELU phases pair with full-size mm2 phases.
    return tiles


def _build(C: int):
    f32, f32r, bf16, f8 = (
        mybir.dt.float32, mybir.dt.float32r, mybir.dt.bfloat16, mybir.dt.float8e4
    )
    DR = mybir.MatmulPerfMode.DoubleRow
    nc = bacc.Bacc("TRN2", target_bir_lowering=False, debug=False, num_devices=8)
    XH = nc.dram_tensor("XH", [P, DP, 2, C], f8, kind="ExternalInput").ap()
    XL = nc.dram_tensor("XL", [P, DP, 2, C], f8, kind="ExternalInput").ap()
    W1H = nc.dram_tensor("W1H", [P, NJ, DP, 2, P], f8, kind="ExternalInput").ap()
    W1L = nc.dram_tensor("W1L", [P, NJ, DP, 2, P], f8, kind="ExternalInput").ap()
    W2H = nc.dram_tensor("W2H", [P, NK, KP, 2, P], f8, kind="ExternalInput").ap()
    W2L = nc.dram_tensor("W2L", [P, NK, KP, 2, P], f8, kind="ExternalInput").ap()
    B2B = nc.dram_tensor("B2B", [P, NK, 2, P], f8, kind="ExternalInput").ap()
    b1 = nc.dram_tensor("b1", [P, NJ], f32, kind="ExternalInput").ap()
    lg = nc.dram_tensor("lg", [P, NJ], f32, kind="ExternalInput").ap()
    lb = nc.dram_tensor("lb", [P, NJ], f32, kind="ExternalInput").ap()
    outT = nc.dram_tensor("outT", [NK, P, C], bf16, kind="ExternalOutput").ap()

    Gelu = mybir.ActivationFunctionType.Gelu
    Ident = mybir.ActivationFunctionType.Identity
    Sqrt = mybir.ActivationFunctionType.Sqrt

    with tile.TileContext(nc) as tc:
        with (
            tc.tile_pool(name="const", bufs=1) as constp,
            tc.tile_pool(name="wp", bufs=1) as wp,
            tc.tile_pool(name="xp", bufs=1) as xp,
            tc.tile_pool(name="hp", bufs=2) as hp,
            tc.tile_pool(name="hxp", bufs=2) as hxp,
            tc.tile_pool(name="sqp", bufs=2) as sqp,
            tc.tile_pool(name="op", bufs=2) as op,
            tc.tile_pool(name="statp", bufs=1) as statp,
            tc.tile_pool(name="ps_mm", bufs=3, space="PSUM") as ps_mm,
            tc.tile_pool(name="ps_acc", bufs=1, space="PSUM") as ps_acc,
            tc.tile_pool(name="ps_bc", bufs=1, space="PSUM") as ps_bc,
        ):
            b1s = constp.tile([P, NJ], f32)
            nc.sync.dma_start(b1s[:], b1[:])
            lgs = constp.tile([P, NJ], f32)
            nc.sync.dma_start(lgs[:], lg[:])
            lbs = constp.tile([P, NJ], f32)
            nc.sync.dma_start(lbs[:], lb[:])
            ones_f = constp.tile([P, 32], f32)
            nc.any.memset(ones_f[:], 1.0)
            ones_bf = constp.tile([P, 32], bf16)  # lhsT for S partition-sums
            nc.vector.tensor_copy(ones_bf[:], ones_f[:])
            ones_q = constp.tile([P, 2, 32], f8)  # lhsT for Q paired DR sums
            nc.any.memset(ones_q[:], 1.0)
            ones_b2 = constp.tile([P, 2, TT], f8)  # rhs for the b2-fold matmul
            nc.any.memset(ones_b2[:], 1.0 / 16.0)
            oner_f = constp.tile([1, P], f32)
            nc.any.memset(oner_f[:], 1.0)
            oner_c = constp.tile([1, P], f32r)    # lhsT for partition-broadcasts
            nc.vector.tensor_copy(oner_c[:], oner_f[:])
            eps_t = constp.tile([1, 1], f32)
            nc.any.memset(eps_t[:], LN_EPS)

            # fp8 weights, SBUF-resident for the whole kernel. DMAs for the
            # first-tile W1 slices are emitted inside the first j-loop so
            # compute starts immediately; W2/B2B stream in behind them.
            w1h = wp.tile([P, NJ, DP, 2, P], f8)
            w1l = wp.tile([P, NJ, DP, 2, P], f8)
            w2h = wp.tile([P, NK, KP, 2, P], f8)
            w2l = wp.tile([P, NK, KP, 2, P], f8)
            b2b = wp.tile([P, NK, 2, P], f8)

            w_cursor = [0]

            def emit_w(n):
                # next n weight chunk-pairs: W1 j-chunks, then W2 k-chunks,
                # then B2B. Each chunk is contiguous per partition.
                for _ in range(n):
                    c = w_cursor[0]
                    w_cursor[0] += 1
                    if c < NJ:
                        nc.sync.dma_start(w1h[:, c], W1H[:, c])
                        nc.sync.dma_start(w1l[:, c], W1L[:, c])
                    elif c < 2 * NJ:
                        k = c - NJ
                        nc.sync.dma_start(w2h[:, k], W2H[:, k])
                        nc.sync.dma_start(w2l[:, k], W2L[:, k])
                    elif c == 2 * NJ:
                        nc.sync.dma_start(b2b[:], B2B[:])

            def emit_mm2(h_hi, h_lo, t0, tt):
                # 3-pass compensated fp8 mm2 + b2-fold, evict on DVE
                for k in range(NK):
                    pm = ps_mm.tile([P, TT], f32, tag="mm", name="mm2")[:, :tt]
                    for pi, (wt, ht) in enumerate(
                        ((w2h, h_hi), (w2h, h_lo), (w2l, h_hi))
                    ):
                        for kp in range(KP):
                            nc.tensor.matmul(
                                pm[:],
                                wt[:, k, kp, :, :],
                                ht[:, kp, :, :tt],
                                start=(pi == 0 and kp == 0),
                                stop=False,
                                perf_mode=DR,
                            )
                    nc.tensor.matmul(
                        pm[:], b2b[:, k, :, :], ones_b2[:, :, :tt],
                        start=False, stop=True, perf_mode=DR,
                    )
                    ot = op.tile(
                        [P, tt], bf16, tag=f"out{tt}", name="out",
                        bufs=(2 if tt == TT else 1),
                    )
                    nc.vector.tensor_scalar_mul(ot[:], pm[:], 1.0 / SW2)
                    nc.sync.dma_start(outT[k, :, t0 : t0 + tt], ot[:])
                    yield k

            def emit_norm_gelu(h, h_hi, h_lo, j, tt):
                # normalize (DVE bf16) + GELU twice (ACT: fp8 h_hi, bf16 in
                # place) + h_lo residual (DVE)
                jp, pl = j // 2, j % 2
                hj = h[:, j, :tt]
                nc.vector.tensor_mul(hj, hj, a_sb[:, :tt])
                nc.vector.tensor_sub(hj, hj, b_sb[:, :tt])
                nc.scalar.activation(
                    h_hi[:, jp, pl, :tt], hj, Gelu,
                    bias=lbs[:, j : j + 1], scale=lgs[:, j : j + 1],
                )
                nc.scalar.activation(
                    hj, hj, Gelu, bias=lbs[:, j : j + 1], scale=lgs[:, j : j + 1]
                )
                nc.vector.tensor_sub(h_lo[:, jp, pl, :tt], hj, h_hi[:, jp, pl, :tt])

            prev = None
            tile_i = 0
            for t0, tt in _t_tiles(C):
                xh = xp.tile([P, DP, 2, TT], f8, tag="xh", name="xh")
                xl = xp.tile([P, DP, 2, TT], f8, tag="xl", name="xl")
                nc.sync.dma_start(xh[:, :, :, :tt], XH[:, :, :, t0 : t0 + tt])
                nc.sync.dma_start(xl[:, :, :, :tt], XL[:, :, :, t0 : t0 + tt])
                h = hp.tile(
                    [P, NJ, tt], bf16, tag=f"h{tt}", name="h",
                    bufs=(2 if tt == TT else 1),
                )
                h_hi = hxp.tile(
                    [P, KP, 2, tt], f8, tag=f"hh{tt}", name="h_hi",
                    bufs=(2 if tt == TT else 1),
                )
                h_lo = hxp.tile(
                    [P, KP, 2, tt], f8, tag=f"hl{tt}", name="h_lo",
                    bufs=(2 if tt == TT else 1),
                )
                s_ps = ps_acc.tile([32, TT], f32, tag="sacc", name="sacc")[:, :tt]
                q_ps = ps_acc.tile([32, TT], f32, tag="qacc", name="qacc")[:, :tt]

                # ---- mm1 (3-pass fp8 DR); S/Q ones-matmuls deferred so the
                # PE never waits on the ACT evict / DVE square chain ----
                pend_s = None
                pend_q = None
                sq = None
                if tile_i == 0:
                    emit_w(4)  # W1 j=0..3 ahead of the first chains
                for j in range(NJ):
                    if tile_i == 0 and j < 12:
                        emit_w(1)  # W1 j=4..15
                    elif tile_i == 0 and j >= 12:
                        emit_w(2)  # W2 k=0..7 behind W1
                    elif tile_i == 1:
                        emit_w(1)  # W2 k=8..15 + B2B
                    pm = ps_mm.tile([P, TT], f32, tag="mm", name="mm1")[:, :tt]
                    for pi, (wt, xt) in enumerate(((w1h, xh), (w1h, xl), (w1l, xh))):
                        for dp in range(DP):
                            nc.tensor.matmul(
                                pm[:],
                                wt[:, j, dp, :, :],
                                xt[:, dp, :, :tt],
                                start=(pi == 0 and dp == 0),
                                stop=(pi == 2 and dp == DP - 1),
                                perf_mode=DR,
                            )
                    nc.scalar.activation(
                        h[:, j, :], pm[:], Ident,
                        bias=b1s[:, j : j + 1], scale=1.0 / (SX * SW1),
                    )
                    if j % 2 == 0:
                        sq = sqp.tile([P, 2, TT], f8, tag="sq", name="sq")
                    nc.vector.tensor_mul(sq[:, j % 2, :tt], h[:, j, :], h[:, j, :])
                    if pend_s is not None:
                        jj = pend_s
                        nc.tensor.matmul(
                            s_ps[:], ones_bf[:], h[:, jj, :],
                            start=(jj == 0), stop=(jj == NJ - 1),
                        )
                    pend_s = j
                    if j % 2 == 1:
                        if pend_q is not None:
                            jp, sqt = pend_q
                            nc.tensor.matmul(
                                q_ps[:], ones_q[:], sqt[:, :, :tt],
                                start=(jp == 0), stop=(jp == NJ // 2 - 1),
                                perf_mode=DR,
                            )
                        pend_q = (j // 2, sq)
                jj = pend_s
                nc.tensor.matmul(
                    s_ps[:], ones_bf[:], h[:, jj, :],
                    start=(jj == 0), stop=(jj == NJ - 1),
                )
                jp, sqt = pend_q
                nc.tensor.matmul(
                    q_ps[:], ones_q[:], sqt[:, :, :tt],
                    start=(jp == 0), stop=(jp == NJ // 2 - 1),
                    perf_mode=DR,
                )
                tile_i += 1

                # ---- LN stats -> A=rstd, B=mu*rstd rows, broadcast, bf16 ----
                mu = statp.tile([1, TT], f32, tag="mu", name="mu")[:, :tt]
                nc.vector.tensor_scalar_mul(mu[:], s_ps[0:1, :], 1.0 / H)
                tmp = statp.tile([1, TT], f32, tag="tmp", name="tmp")[:, :tt]
                nc.vector.tensor_scalar_mul(tmp[:], q_ps[0:1, :], 1.0 / H)
                tmp2 = statp.tile([1, TT], f32, tag="tmp2", name="tmp2")[:, :tt]
                nc.vector.tensor_mul(tmp2[:], mu[:], mu[:])
                nc.vector.tensor_sub(tmp[:], tmp[:], tmp2[:])          # var
                nc.scalar.activation(tmp2[:], tmp[:], Sqrt, bias=eps_t[:])  # std
                nc.vector.reciprocal(tmp[:], tmp2[:])                  # rstd
                a_row = statp.tile([1, TT], f32r, tag="a_row", name="a_row")[:, :tt]
                nc.vector.tensor_copy(a_row[:], tmp[:])
                b_row = statp.tile([1, TT], f32r, tag="b_row", name="b_row")[:, :tt]
                nc.vector.tensor_mul(b_row[:], mu[:], tmp[:])
                a_bc = ps_bc.tile([P, TT], f32, tag="a_bc", name="a_bc")[:, :tt]
                nc.tensor.matmul(a_bc[:], oner_c[:], a_row[:], start=True, stop=True)
                b_bc = ps_bc.tile([P, TT], f32, tag="b_bc", name="b_bc")[:, :tt]
                nc.tensor.matmul(b_bc[:], oner_c[:], b_row[:], start=True, stop=True)
                a_sb = statp.tile([P, TT], bf16, tag="a_sb", name="a_sb", bufs=2)
                nc.vector.tensor_copy(a_sb[:, :tt], a_bc[:])
                b_sb = statp.tile([P, TT], bf16, tag="b_sb", name="b_sb", bufs=2)
                nc.vector.tensor_copy(b_sb[:, :tt], b_bc[:])

                # ---- previous tile's mm2 on the PE, interleaved with this
                # tile's normalize+GELU+hi/lo split on DVE/ACT ----
                if prev is not None:
                    ph_hi, ph_lo, pt0, ptt = prev
                    for k in emit_mm2(ph_hi, ph_lo, pt0, ptt):
                        emit_norm_gelu(h, h_hi, h_lo, k, tt)
                else:
                    for j in range(NJ):
                        emit_norm_gelu(h, h_hi, h_lo, j, tt)
                prev = (h_hi, h_lo, t0, tt)

            emit_w(2 * NJ + 1 - w_cursor[0])  # any chunks not yet emitted
            for _ in emit_mm2(*prev):
                pass

    nc.compile()
    return nc


def _route(x64, Wg64, bg64):
    """Host gating: per-token top-2 expert ids and renormalized weights."""
    logits = x64 @ Wg64 + bg64                      # [N, E] fp64
    order = np.argsort(-logits, axis=1, kind="stable")[:, :TOPK]
    l0 = np.take_along_axis(logits, order, axis=1)  # [N, 2] descending
    w0 = 1.0 / (1.0 + np.exp(l0[:, 1] - l0[:, 0]))
    w = np.stack([w0, 1.0 - w0], axis=1)
    return order, w


def _split8(a):
    hi = a.astype(F8)
    lo = (a - hi.astype(np.float32)).astype(F8)
    return hi, lo


def kernel(x, W1, b1, ln_g, ln_b, W2, b2, Wg, bg):
    x = np.ascontiguousarray(np.asarray(x, dtype=np.float32))
    W1 = np.asarray(W1, dtype=np.float32)
    b1 = np.asarray(b1, dtype=np.float32)
    ln_g = np.asarray(ln_g, dtype=np.float32)
    ln_b = np.asarray(ln_b, dtype=np.float32)
    W2 = np.asarray(W2, dtype=np.float32)
    b2 = np.asarray(b2, dtype=np.float32)
    Wg = np.asarray(Wg, dtype=np.float32)
    bg = np.asarray(bg, dtype=np.float32)
    N = x.shape[0]

    order, w = _route(x.astype(np.float64), Wg.astype(np.float64), bg.astype(np.float64))

    tok_idx, tok_w = [], []
    for e in range(E):
        sel = np.nonzero((order[:, 0] == e) | (order[:, 1] == e))[0]
        we = np.where(order[sel, 0] == e, w[sel, 0], w[sel, 1]).astype(np.float32)
        tok_idx.append(sel)
        tok_w.append(we)
    C = max(GRAN, int(-(-max(len(s) for s in tok_idx) // GRAN)) * GRAN)

    if C not in _kernel_cache:
        _kernel_cache[C] = _build(C)
    nc = _kernel_cache[C]

    in_maps = []
    for e in range(E):
        idx = np.zeros(C, dtype=np.int64)
        idx[: len(tok_idx[e])] = tok_idx[e]
        xg = x[idx] * SX                              # [C, D]
        xh, xl = _split8(xg)
        # [C, D] -> [P, DP, 2, C]
        xh_d = np.ascontiguousarray(xh.reshape(C, DP, 2, P).transpose(3, 1, 2, 0))
        xl_d = np.ascontiguousarray(xl.reshape(C, DP, 2, P).transpose(3, 1, 2, 0))
        w1h, w1l = _split8(W1[e] * SW1)               # [D, H]
        w1h_d = np.ascontiguousarray(
            w1h.reshape(DP, 2, P, NJ, P).transpose(2, 3, 0, 1, 4)
        )
        w1l_d = np.ascontiguousarray(
            w1l.reshape(DP, 2, P, NJ, P).transpose(2, 3, 0, 1, 4)
        )
        w2h, w2l = _split8(W2[e] * SW2)               # [H, H]
        w2h_d = np.ascontiguousarray(
            w2h.reshape(KP, 2, P, NK, P).transpose(2, 3, 0, 1, 4)
        )
        w2l_d = np.ascontiguousarray(
            w2l.reshape(KP, 2, P, NK, P).transpose(2, 3, 0, 1, 4)
        )
        # b2 fold: contribution = sum_{p,pl} B2B[p,k,pl,m] * (1/16)
        #        = 128*(q0+q1)/16 = 8*(32*b2) = SW2*b2
        q0, q1 = _split8(32.0 * b2[e])                # [H]
        b2b = np.stack([q0, q1], axis=0).reshape(2, NK, P).transpose(1, 0, 2)
        b2b_d = np.ascontiguousarray(
            np.broadcast_to(b2b[None], (P, NK, 2, P)).astype(F8)
        )
        in_maps.append(
            {
                "XH": xh_d,
                "XL": xl_d,
                "W1H": w1h_d,
                "W1L": w1l_d,
                "W2H": w2h_d,
                "W2L": w2l_d,
                "B2B": b2b_d,
                "b1": np.ascontiguousarray(b1[e].reshape(NJ, P).T),
                "lg": np.ascontiguousarray(ln_g[e].reshape(NJ, P).T),
                "lb": np.ascontiguousarray(ln_b[e].reshape(NJ, P).T),
            }
        )

    results = _run(C, nc, in_maps)

    y = np.zeros((N, H), dtype=np.float32)
    for e in range(E):
        cnt = len(tok_idx[e])
        eoT = results[e]["outT"].reshape(H, C).astype(np.float32)
        y[tok_idx[e]] += tok_w[e][:, None] * eoT[:, :cnt].T
    return y


_neff_cache: dict[int, str] = {}


def _run(C, nc, in_maps):
    if axon_active():
        # PJRT path; NEFF compile is cached by libneuronxla.
        return run_bass_kernel_spmd(nc, in_maps, core_ids=list(range(E))).results
    # Native path: compile once per capacity, then execute the cached NEFF.
    from concourse.bass_utils import compile_bass_kernel, run_neff

    if C not in _neff_cache:
        _neff_cache[C] = compile_bass_kernel(nc, tempfile.mkdtemp())
    out_maps = [{"outT": np.zeros((NK, P, C), dtype=BF)} for _ in range(E)]
    in_maps = [m.copy() for m in in_maps]
    if nc.partition_id_tensor:
        for core_id, m in enumerate(in_maps):
            m[nc.partition_id_tensor.name] = np.array([[core_id]], dtype=np.uint32)
    return run_neff(
        _neff_cache[C],
        in_maps,
        out_maps,
        core_ids=list(range(E)),
        has_collectives=False,
    )


# revision 9
# speedup vs baseline: 1.6664x; 1.1435x over previous
# MoE (top-2 of 8 experts) kernel for 8 Trainium2 NeuronCores.
#
# Strategy: expert-parallel sparse routing with fp8 DoubleRow matmuls.
# Host computes the gating network and per-expert token lists; core e runs
# expert e's FFN (x@W1+b1 -> LayerNorm -> erf-GELU -> @W2+b2) on its routed
# tokens. Both matmuls run as fp8(e4m3) DoubleRow (2 k-planes per
# instruction, 0.5 cyc/row = 4x the f32r MAC rate) with hi/lo error
# compensation: A@B ~= Ah@Bh + Al@Bh + Ah@Bl where Ah=fp8(A), Al=fp8(A-Ah).
# All weights live in SBUF (12MB fp8), loaded once in contiguous per-chunk
# DMAs. LayerNorm S-sums are folded into mm1 via a W1-column-sum lhsT row;
# Q-sums use a paired-fp8 DoubleRow ones-matmul on h^2; per-token stats
# broadcast across partitions with K=1 f32r matmuls, emitted mid-mm2 so the
# PE never waits on the stats chain. b2 is folded into the mm2 PSUM chain
# via a constant fp8 matmul so the mm2 evict is a single DVE
# tensor_scalar_mul (keeps ACT free for the GELUs). The first tile's
# normalize/GELU backlog is drip-fed through the second tile's mm1 loop.

import tempfile

import ml_dtypes
import numpy as np

import concourse.bacc as bacc
import concourse.mybir as mybir
import concourse.tile as tile
from concourse._compat import axon_active
from concourse.bass_utils import run_bass_kernel_spmd

P = 128
D, H, E, TOPK = 1024, 2048, 8, 2
DP, KP, NJ, NK = D // 256, H // 256, H // P, H // P  # 4, 8, 16, 16
LN_EPS = 1e-5
TT = 512           # main token tile
GRAN = 128         # capacity granularity
SX, SW1, SW2 = 16.0, 256.0, 256.0   # fp8 pre-quantization scales
SW1S = 32.0        # scale for the W1 column-sum row (S-fold)
F8 = ml_dtypes.float8_e4m3
BF = ml_dtypes.bfloat16

_kernel_cache: dict[int, object] = {}


def _t_tiles(C):
    tiles, t0 = [], 0
    while t0 < C:
        tt = TT if C - t0 >= TT else C - t0
        tiles.append((t0, tt))
        t0 += tt
    # Tail tile last: its cheap mm2 is the only un-overlapped one, and
    # full-size norm/GELU phases pair with full-size mm2 phases.
    return tiles


def _build(C: int):
    f32, f32r, bf16, f8 = (
        mybir.dt.float32, mybir.dt.float32r, mybir.dt.bfloat16, mybir.dt.float8e4
    )
    DR = mybir.MatmulPerfMode.DoubleRow
    Mul, Add = mybir.AluOpType.mult, mybir.AluOpType.add
    nc = bacc.Bacc("TRN2", target_bir_lowering=False, debug=False, num_devices=8)
    XH = nc.dram_tensor("XH", [P, DP, 2, C], f8, kind="ExternalInput").ap()
    XL = nc.dram_tensor("XL", [P, DP, 2, C], f8, kind="ExternalInput").ap()
    W1H = nc.dram_tensor("W1H", [P, NJ, DP, 2, P], f8, kind="ExternalInput").ap()
    W1L = nc.dram_tensor("W1L", [P, NJ, DP, 2, P], f8, kind="ExternalInput").ap()
    W1SH = nc.dram_tensor("W1SH", [P, DP, 2, 32], f8, kind="ExternalInput").ap()
    W1SL = nc.dram_tensor("W1SL", [P, DP, 2, 32], f8, kind="ExternalInput").ap()
    SB1H = nc.dram_tensor("SB1H", [1, 1], f32, kind="ExternalInput").ap()
    W2H = nc.dram_tensor("W2H", [P, NK, KP, 2, P], f8, kind="ExternalInput").ap()
    W2L = nc.dram_tensor("W2L", [P, NK, KP, 2, P], f8, kind="ExternalInput").ap()
    B2B = nc.dram_tensor("B2B", [P, NK, 2, P], f8, kind="ExternalInput").ap()
    b1 = nc.dram_tensor("b1", [P, NJ], f32, kind="ExternalInput").ap()
    lg = nc.dram_tensor("lg", [P, NJ], f32, kind="ExternalInput").ap()
    lb = nc.dram_tensor("lb", [P, NJ], f32, kind="ExternalInput").ap()
    outT = nc.dram_tensor("outT", [NK, P, C], bf16, kind="ExternalOutput").ap()

    Gelu = mybir.ActivationFunctionType.Gelu
    Sqrt = mybir.ActivationFunctionType.Sqrt
    Ident = mybir.ActivationFunctionType.Identity

    with tile.TileContext(nc) as tc:
        with (
            tc.tile_pool(name="const", bufs=1) as constp,
            tc.tile_pool(name="wp", bufs=1) as wp,
            tc.tile_pool(name="xp", bufs=1) as xp,
            tc.tile_pool(name="hp", bufs=2) as hp,
            tc.tile_pool(name="hxp", bufs=2) as hxp,
            tc.tile_pool(name="sqp", bufs=2) as sqp,
            tc.tile_pool(name="op", bufs=2) as op,
            tc.tile_pool(name="statp", bufs=1) as statp,
            tc.tile_pool(name="ps_mm", bufs=3, space="PSUM") as ps_mm,
            tc.tile_pool(name="ps_acc", bufs=1, space="PSUM") as ps_acc,
            tc.tile_pool(name="ps_bc", bufs=1, space="PSUM") as ps_bc,
        ):
            b1s = constp.tile([P, NJ], f32)
            nc.sync.dma_start(b1s[:], b1[:])
            lgs = constp.tile([P, NJ], f32)
            nc.sync.dma_start(lgs[:], lg[:])
            lbs = constp.tile([P, NJ], f32)
            nc.sync.dma_start(lbs[:], lb[:])
            w1sh = constp.tile([P, DP, 2, 32], f8)
            nc.sync.dma_start(w1sh[:], W1SH[:])
            w1sl = constp.tile([P, DP, 2, 32], f8)
            nc.sync.dma_start(w1sl[:], W1SL[:])
            sb1h = constp.tile([1, 1], f32)
            nc.sync.dma_start(sb1h[:], SB1H[:])
            ones_q = constp.tile([P, 2, 32], f8)   # lhsT for Q paired DR sums
            nc.any.memset(ones_q[:], 1.0)
            ones_b2 = constp.tile([P, 2, TT], f8)  # rhs for the b2-fold matmul
            nc.any.memset(ones_b2[:], 1.0 / 16.0)
            oner_f = constp.tile([1, P], f32)
            nc.any.memset(oner_f[:], 1.0)
            oner_c = constp.tile([1, P], f32r)     # lhsT for partition-broadcasts
            nc.vector.tensor_copy(oner_c[:], oner_f[:])
            eps_t = constp.tile([1, 1], f32)
            nc.any.memset(eps_t[:], LN_EPS)

            # fp8 weights, SBUF-resident for the whole kernel, streamed in
            # contiguous per-chunk DMAs staged around the first two tiles.
            w1h = wp.tile([P, NJ, DP, 2, P], f8)
            w1l = wp.tile([P, NJ, DP, 2, P], f8)
            w2h = wp.tile([P, NK, KP, 2, P], f8)
            w2l = wp.tile([P, NK, KP, 2, P], f8)
            b2b = wp.tile([P, NK, 2, P], f8)

            w_cursor = [0]

            def emit_w(n):
                # next n weight chunk-pairs: W1 j-chunks, then W2 k-chunks,
                # then B2B. Each chunk is contiguous per partition.
                for _ in range(n):
                    c = w_cursor[0]
                    w_cursor[0] += 1
                    if c < NJ:
                        nc.sync.dma_start(w1h[:, c], W1H[:, c])
                        nc.sync.dma_start(w1l[:, c], W1L[:, c])
                    elif c < 2 * NJ:
                        k = c - NJ
                        nc.sync.dma_start(w2h[:, k], W2H[:, k])
                        nc.sync.dma_start(w2l[:, k], W2L[:, k])
                    elif c == 2 * NJ:
                        nc.sync.dma_start(b2b[:], B2B[:])

            def emit_mm2(h_hi, h_lo, t0, tt, mid=None, post=None):
                # 3-pass compensated fp8 mm2 + b2-fold, evict on DVE.
                # mid() runs after chain 5 (the next tile's stats-broadcast
                # matmuls); post(j) runs twice per chain from chain 6 (the
                # next tile's normalize/GELU work).
                step = 0
                for k in range(NK):
                    pm = ps_mm.tile([P, TT], f32, tag="mm", name="mm2")[:, :tt]
                    for pi, (wt, ht) in enumerate(
                        ((w2h, h_hi), (w2h, h_lo), (w2l, h_hi))
                    ):
                        for kp in range(KP):
                            nc.tensor.matmul(
                                pm[:],
                                wt[:, k, kp, :, :],
                                ht[:, kp, :, :tt],
                                start=(pi == 0 and kp == 0),
                                stop=False,
                                perf_mode=DR,
                            )
                    nc.tensor.matmul(
                        pm[:], b2b[:, k, :, :], ones_b2[:, :, :tt],
                        start=False, stop=True, perf_mode=DR,
                    )
                    ot = op.tile([P, tt], bf16, tag=f"out{tt}", name="out")
                    nc.vector.tensor_scalar_mul(ot[:], pm[:], 1.0 / SW2)
                    nc.sync.dma_start(outT[k, :, t0 : t0 + tt], ot[:])
                    if k == 5 and mid is not None:
                        mid()
                    if k >= 6 and post is not None:
                        for _ in range(2):
                            if step < NJ:
                                post(step)
                                step += 1
                while post is not None and step < NJ:
                    post(step)
                    step += 1

            prev = None
            backlog = []
            for tile_i, (t0, tt) in enumerate(_t_tiles(C)):
                if tile_i == 0:
                    emit_w(1)  # W1 j=0 ahead of x so the first chain starts fast
                xh = xp.tile([P, DP, 2, TT], f8, tag="xh", name="xh")
                xl = xp.tile([P, DP, 2, TT], f8, tag="xl", name="xl")
                nc.sync.dma_start(xh[:, :, :, :tt], XH[:, :, :, t0 : t0 + tt])
                nc.sync.dma_start(xl[:, :, :, :tt], XL[:, :, :, t0 : t0 + tt])
                if tile_i == 0:
                    emit_w(3)  # W1 j=1..3
                h = hp.tile(
                    [P, NJ, tt], bf16, tag=f"h{tt}", name="h",
                    bufs=(2 if tt == TT else 1),
                )
                h_hi = hxp.tile(
                    [P, KP, 2, tt], f8, tag=f"hh{tt}", name="h_hi",
                    bufs=(2 if tt == TT else 1),
                )
                h_lo = hxp.tile(
                    [P, KP, 2, tt], f8, tag=f"hl{tt}", name="h_lo",
                    bufs=(2 if tt == TT else 1),
                )
                s_ps = ps_acc.tile([32, TT], f32, tag="sacc", name="sacc")[:, :tt]
                q_ps = ps_acc.tile([32, TT], f32, tag="qacc", name="qacc")[:, :tt]

                # ---- mm1 (3-pass fp8 DR); Q ones-matmuls deferred one pair
                # so the PE never waits on the ACT evict / DVE square chain;
                # tile0's norm/GELU backlog drip-fed through tile1's loop ----
                pend_q = None
                sq = None
                for j in range(NJ):
                    if tile_i == 0 and j < 12:
                        emit_w(1)  # W1 j=4..15
                    elif tile_i == 0 and j >= 12:
                        emit_w(2)  # W2 k=0..7 behind W1
                    elif tile_i == 1:
                        emit_w(1)  # W2 k=8..15 + B2B
                    pm = ps_mm.tile([P, TT], f32, tag="mm", name="mm1")[:, :tt]
                    for pi, (wt, xt) in enumerate(((w1h, xh), (w1h, xl), (w1l, xh))):
                        for dp in range(DP):
                            nc.tensor.matmul(
                                pm[:],
                                wt[:, j, dp, :, :],
                                xt[:, dp, :, :tt],
                                start=(pi == 0 and dp == 0),
                                stop=(pi == 2 and dp == DP - 1),
                                perf_mode=DR,
                            )
                    nc.scalar.activation(
                        h[:, j, :], pm[:], Ident,
                        bias=b1s[:, j : j + 1], scale=1.0 / (SX * SW1),
                    )
                    if j % 2 == 0:
                        sq = sqp.tile([P, 2, TT], f8, tag="sq", name="sq")
                    nc.vector.tensor_mul(sq[:, j % 2, :tt], h[:, j, :], h[:, j, :])
                    if j % 2 == 1:
                        if pend_q is not None:
                            jp, sqt = pend_q
                            nc.tensor.matmul(
                                q_ps[:], ones_q[:], sqt[:, :, :tt],
                                start=(jp == 0), stop=(jp == NJ // 2 - 1),
                                perf_mode=DR,
                            )
                        pend_q = (j // 2, sq)
                    if backlog:
                        backlog.pop(0)()
                while backlog:  # finish tile0's backlog before mm2(0) reads h
                    backlog.pop(0)()
                # S-fold: the W1-column-sum row, 3-pass DR into s_ps
                for pi, (wt, xt) in enumerate(((w1sh, xh), (w1sh, xl), (w1sl, xh))):
                    for dp in range(DP):
                        nc.tensor.matmul(
                            s_ps[:],
                            wt[:, dp, :, :],
                            xt[:, dp, :, :tt],
                            start=(pi == 0 and dp == 0),
                            stop=(pi == 2 and dp == DP - 1),
                            perf_mode=DR,
                        )
                jp, sqt = pend_q
                nc.tensor.matmul(
                    q_ps[:], ones_q[:], sqt[:, :, :tt],
                    start=(jp == 0), stop=(jp == NJ // 2 - 1),
                    perf_mode=DR,
                )

                # ---- LN stats (DVE/ACT only; broadcasts happen mid-mm2) ----
                mu = statp.tile([1, TT], f32, tag="mu", name="mu")[:, :tt]
                nc.vector.tensor_scalar(
                    mu[:], s_ps[0:1, :], 1.0 / (SX * SW1S * H), sb1h[:], Mul, Add
                )
                tmp = statp.tile([1, TT], f32, tag="tmp", name="tmp")[:, :tt]
                nc.vector.tensor_scalar_mul(tmp[:], q_ps[0:1, :], 1.0 / H)
                tmp2 = statp.tile([1, TT], f32, tag="tmp2", name="tmp2")[:, :tt]
                nc.vector.tensor_mul(tmp2[:], mu[:], mu[:])
                nc.vector.tensor_sub(tmp[:], tmp[:], tmp2[:])          # var
                nc.scalar.activation(tmp2[:], tmp[:], Sqrt, bias=eps_t[:])  # std
                nc.vector.reciprocal(tmp[:], tmp2[:])                  # rstd
                a_row = statp.tile([1, TT], f32r, tag="a_row", name="a_row")[:, :tt]
                nc.vector.tensor_copy(a_row[:], tmp[:])
                b_row = statp.tile([1, TT], f32r, tag="b_row", name="b_row")[:, :tt]
                nc.vector.tensor_mul(b_row[:], mu[:], tmp[:])

                a_sb = statp.tile([P, TT], bf16, tag="a_sb", name="a_sb", bufs=2)
                b_sb = statp.tile([P, TT], bf16, tag="b_sb", name="b_sb", bufs=2)

                def emit_bc(a_row=a_row, b_row=b_row, a_sb=a_sb, b_sb=b_sb, tt=tt):
                    a_bc = ps_bc.tile([P, TT], f32, tag="a_bc", name="a_bc")[:, :tt]
                    nc.tensor.matmul(
                        a_bc[:], oner_c[:], a_row[:], start=True, stop=True
                    )
                    b_bc = ps_bc.tile([P, TT], f32, tag="b_bc", name="b_bc")[:, :tt]
                    nc.tensor.matmul(
                        b_bc[:], oner_c[:], b_row[:], start=True, stop=True
                    )
                    nc.vector.tensor_copy(a_sb[:, :tt], a_bc[:])
                    nc.vector.tensor_copy(b_sb[:, :tt], b_bc[:])

                def emit_norm_gelu(
                    j, h=h, h_hi=h_hi, h_lo=h_lo, a_sb=a_sb, b_sb=b_sb, tt=tt
                ):
                    # normalize (DVE bf16) + GELU twice (ACT: fp8 h_hi, bf16
                    # in place) + h_lo residual (DVE)
                    jp, pl = j // 2, j % 2
                    hj = h[:, j, :tt]
                    nc.vector.tensor_mul(hj, hj, a_sb[:, :tt])
                    nc.vector.tensor_sub(hj, hj, b_sb[:, :tt])
                    nc.scalar.activation(
                        h_hi[:, jp, pl, :tt], hj, Gelu,
                        bias=lbs[:, j : j + 1], scale=lgs[:, j : j + 1],
                    )
                    nc.scalar.activation(
                        hj, hj, Gelu, bias=lbs[:, j : j + 1], scale=lgs[:, j : j + 1]
                    )
                    nc.vector.tensor_sub(
                        h_lo[:, jp, pl, :tt], hj, h_hi[:, jp, pl, :tt]
                    )

                # ---- previous tile's mm2 on the PE, with this tile's
                # broadcasts at chain 5 and norm/GELU from chain 6 ----
                if prev is not None:
                    emit_mm2(*prev, mid=emit_bc, post=emit_norm_gelu)
                else:
                    backlog.append(emit_bc)
                    backlog.extend(
                        (lambda j=j, f=emit_norm_gelu: f(j)) for j in range(NJ)
                    )
                prev = (h_hi, h_lo, t0, tt)

            emit_w(2 * NJ + 1 - w_cursor[0])
            for fn in backlog:
                fn()
            emit_mm2(*prev)

    nc.compile()
    return nc


def _route(x64, Wg64, bg64):
    """Host gating: per-token top-2 expert ids and renormalized weights."""
    logits = x64 @ Wg64 + bg64                      # [N, E] fp64
    order = np.argsort(-logits, axis=1, kind="stable")[:, :TOPK]
    l0 = np.take_along_axis(logits, order, axis=1)  # [N, 2] descending
    w0 = 1.0 / (1.0 + np.exp(l0[:, 1] - l0[:, 0]))
    w = np.stack([w0, 1.0 - w0], axis=1)
    return order, w


def _split8(a):
    hi = a.astype(F8)
    lo = (a - hi.astype(np.float32)).astype(F8)
    return hi, lo


def kernel(x, W1, b1, ln_g, ln_b, W2, b2, Wg, bg):
    x = np.ascontiguousarray(np.asarray(x, dtype=np.float32))
    W1 = np.asarray(W1, dtype=np.float32)
    b1 = np.asarray(b1, dtype=np.float32)
    ln_g = np.asarray(ln_g, dtype=np.float32)
    ln_b = np.asarray(ln_b, dtype=np.float32)
    W2 = np.asarray(W2, dtype=np.float32)
    b2 = np.asarray(b2, dtype=np.float32)
    Wg = np.asarray(Wg, dtype=np.float32)
    bg = np.asarray(bg, dtype=np.float32)
    N = x.shape[0]

    order, w = _route(x.astype(np.float64), Wg.astype(np.float64), bg.astype(np.float64))

    tok_idx, tok_w = [], []
    for e in range(E):
        sel = np.nonzero((order[:, 0] == e) | (order[:, 1] == e))[0]
        we = np.where(order[sel, 0] == e, w[sel, 0], w[sel, 1]).astype(np.float32)
        tok_idx.append(sel)
        tok_w.append(we)
    C = max(GRAN, int(-(-max(len(s) for s in tok_idx) // GRAN)) * GRAN)

    if C not in _kernel_cache:
        _kernel_cache[C] = _build(C)
    nc = _kernel_cache[C]

    in_maps = []
    for e in range(E):
        idx = np.zeros(C, dtype=np.int64)
        idx[: len(tok_idx[e])] = tok_idx[e]
        xg = x[idx] * SX                              # [C, D]
        xh, xl = _split8(xg)
        # [C, D] -> [P, DP, 2, C]
        xh_d = np.ascontiguousarray(xh.reshape(C, DP, 2, P).transpose(3, 1, 2, 0))
        xl_d = np.ascontiguousarray(xl.reshape(C, DP, 2, P).transpose(3, 1, 2, 0))
        w1h, w1l = _split8(W1[e] * SW1)               # [D, H]
        w1h_d = np.ascontiguousarray(
            w1h.reshape(DP, 2, P, NJ, P).transpose(2, 3, 0, 1, 4)
        )
        w1l_d = np.ascontiguousarray(
            w1l.reshape(DP, 2, P, NJ, P).transpose(2, 3, 0, 1, 4)
        )
        # S-fold: column-sum of W1 (scaled), replicated over 32 lhsT columns
        w1s = W1[e].sum(axis=1) * SW1S                # [D]
        w1sh, w1sl = _split8(w1s)
        w1sh_d = np.ascontiguousarray(np.broadcast_to(
            w1sh.reshape(DP, 2, P).transpose(2, 0, 1)[:, :, :, None], (P, DP, 2, 32)
        ).astype(F8))
        w1sl_d = np.ascontiguousarray(np.broadcast_to(
            w1sl.reshape(DP, 2, P).transpose(2, 0, 1)[:, :, :, None], (P, DP, 2, 32)
        ).astype(F8))
        sb1h_d = np.full((1, 1), b1[e].sum() / H, dtype=np.float32)
        w2h, w2l = _split8(W2[e] * SW2)               # [H, H]
        w2h_d = np.ascontiguousarray(
            w2h.reshape(KP, 2, P, NK, P).transpose(2, 3, 0, 1, 4)
        )
        w2l_d = np.ascontiguousarray(
            w2l.reshape(KP, 2, P, NK, P).transpose(2, 3, 0, 1, 4)
        )
        # b2 fold: contribution = sum_{p,pl} B2B[p,k,pl,m] * (1/16)
        #        = 128*(q0+q1)/16 = 8*(32*b2) = SW2*b2
        q0, q1 = _split8(32.0 * b2[e])                # [H]
        b2f = np.stack([q0, q1], axis=0).reshape(2, NK, P).transpose(1, 0, 2)
        b2b_d = np.ascontiguousarray(
            np.broadcast_to(b2f[None], (P, NK, 2, P)).astype(F8)
        )
        in_maps.append(
            {
                "XH": xh_d,
                "XL": xl_d,
                "W1H": w1h_d,
                "W1L": w1l_d,
                "W1SH": w1sh_d,
                "W1SL": w1sl_d,
                "SB1H": sb1h_d,
                "W2H": w2h_d,
                "W2L": w2l_d,
                "B2B": b2b_d,
                "b1": np.ascontiguousarray(b1[e].reshape(NJ, P).T),
                "lg": np.ascontiguousarray(ln_g[e].reshape(NJ, P).T),
                "lb": np.ascontiguousarray(ln_b[e].reshape(NJ, P).T),
            }
        )

    results = _run(C, nc, in_maps)

    y = np.zeros((N, H), dtype=np.float32)
    for e in range(E):
        cnt = len(tok_idx[e])
        eoT = results[e]["outT"].reshape(H, C).astype(np.float32)
        y[tok_idx[e]] += tok_w[e][:, None] * eoT[:, :cnt].T
    return y


_neff_cache: dict[int, str] = {}


def _run(C, nc, in_maps):
    if axon_active():
        # PJRT path; NEFF compile is cached by libneuronxla.
        return run_bass_kernel_spmd(nc, in_maps, core_ids=list(range(E))).results
    # Native path: compile once per capacity, then execute the cached NEFF.
    from concourse.bass_utils import compile_bass_kernel, run_neff

    if C not in _neff_cache:
        _neff_cache[C] = compile_bass_kernel(nc, tempfile.mkdtemp())
    out_maps = [{"outT": np.zeros((NK, P, C), dtype=BF)} for _ in range(E)]
    in_maps = [m.copy() for m in in_maps]
    if nc.partition_id_tensor:
        for core_id, m in enumerate(in_maps):
            m[nc.partition_id_tensor.name] = np.array([[core_id]], dtype=np.uint32)
    return run_neff(
        _neff_cache[C],
        in_maps,
        out_maps,
        core_ids=list(range(E)),
        has_collectives=False,
    )


# revision 10
# speedup vs baseline: 1.7443x; 1.0468x over previous
# MoE (top-2 of 8 experts) kernel for 8 Trainium2 NeuronCores.
#
# Strategy: expert-parallel sparse routing with fp8 DoubleRow matmuls.
# Host computes the gating network and per-expert token lists; core e runs
# expert e's FFN (x@W1+b1 -> LayerNorm -> erf-GELU -> @W2+b2) on its routed
# tokens. Both matmuls run as fp8(e4m3) DoubleRow (2 k-planes per
# instruction, 0.5 cyc/row = 4x the f32r MAC rate) with hi/lo error
# compensation: A@B ~= Ah@Bh + Al@Bh + Ah@Bl where Ah=fp8(A), Al=fp8(A-Ah).
# All weights live in SBUF (12MB fp8), loaded once in contiguous per-chunk
# DMAs. LayerNorm S-sums are folded into mm1 via a W1-column-sum lhsT row;
# Q-sums use a paired-fp8 DoubleRow ones-matmul on h^2; per-token stats
# broadcast across partitions with K=1 f32r matmuls, emitted mid-mm2 so the
# PE never waits on the stats chain. b2 is folded into the mm2 PSUM chain
# via a constant fp8 matmul so the mm2 evict is a single DVE
# tensor_scalar_mul (keeps ACT free for the GELUs). The first tile's
# normalize/GELU backlog is drip-fed through the second tile's mm1 loop.

import tempfile

import ml_dtypes
import numpy as np

import concourse.bacc as bacc
import concourse.mybir as mybir
import concourse.tile as tile
from concourse._compat import axon_active
from concourse.bass_utils import run_bass_kernel_spmd

P = 128
D, H, E, TOPK = 1024, 2048, 8, 2
DP, KP, NJ, NK = D // 256, H // 256, H // P, H // P  # 4, 8, 16, 16
LN_EPS = 1e-5
TT = 512           # main token tile
GRAN = 128         # capacity granularity
SX, SW1, SW2 = 16.0, 256.0, 256.0   # fp8 pre-quantization scales
SW1S = 32.0        # scale for the W1 column-sum row (S-fold)
F8 = ml_dtypes.float8_e4m3
BF = ml_dtypes.bfloat16

_kernel_cache: dict[int, object] = {}


def _t_tiles(C):
    tiles, t0 = [], 0
    while t0 < C:
        tt = TT if C - t0 >= TT else C - t0
        tiles.append((t0, tt))
        t0 += tt
    # Tail tile last: its cheap mm2 is the only un-overlapped one, and
    # full-size norm/GELU phases pair with full-size mm2 phases.
    return tiles


def _build(C: int):
    f32, f32r, bf16, f8 = (
        mybir.dt.float32, mybir.dt.float32r, mybir.dt.bfloat16, mybir.dt.float8e4
    )
    DR = mybir.MatmulPerfMode.DoubleRow
    Mul, Add = mybir.AluOpType.mult, mybir.AluOpType.add
    nc = bacc.Bacc("TRN2", target_bir_lowering=False, debug=False, num_devices=8)
    XH = nc.dram_tensor("XH", [P, DP, 2, C], f8, kind="ExternalInput").ap()
    XL = nc.dram_tensor("XL", [P, DP, 2, C], f8, kind="ExternalInput").ap()
    W1H = nc.dram_tensor("W1H", [P, NJ, DP, 2, P], f8, kind="ExternalInput").ap()
    W1L = nc.dram_tensor("W1L", [P, NJ, DP, 2, P], f8, kind="ExternalInput").ap()
    W1SH = nc.dram_tensor("W1SH", [P, DP, 2, 32], f8, kind="ExternalInput").ap()
    W1SL = nc.dram_tensor("W1SL", [P, DP, 2, 32], f8, kind="ExternalInput").ap()
    SB1H = nc.dram_tensor("SB1H", [1, 1], f32, kind="ExternalInput").ap()
    W2H = nc.dram_tensor("W2H", [P, NK, KP, 2, P], f8, kind="ExternalInput").ap()
    W2L = nc.dram_tensor("W2L", [P, NK, KP, 2, P], f8, kind="ExternalInput").ap()
    b1 = nc.dram_tensor("b1", [P, NJ], f32, kind="ExternalInput").ap()
    b2 = nc.dram_tensor("b2", [P, NK], f32, kind="ExternalInput").ap()
    lg = nc.dram_tensor("lg", [P, NJ], f32, kind="ExternalInput").ap()
    lb = nc.dram_tensor("lb", [P, NJ], f32, kind="ExternalInput").ap()
    outT = nc.dram_tensor("outT", [NK, P, C], bf16, kind="ExternalOutput").ap()

    Gelu = mybir.ActivationFunctionType.Gelu
    Sqrt = mybir.ActivationFunctionType.Sqrt
    Ident = mybir.ActivationFunctionType.Identity

    with tile.TileContext(nc) as tc:
        with (
            tc.tile_pool(name="const", bufs=1) as constp,
            tc.tile_pool(name="wp", bufs=1) as wp,
            tc.tile_pool(name="xp", bufs=1) as xp,
            tc.tile_pool(name="hp", bufs=2) as hp,
            tc.tile_pool(name="hxp", bufs=2) as hxp,
            tc.tile_pool(name="sqp", bufs=2) as sqp,
            tc.tile_pool(name="op", bufs=2) as op,
            tc.tile_pool(name="statp", bufs=1) as statp,
            tc.tile_pool(name="ps_mm", bufs=4, space="PSUM") as ps_mm,
            tc.tile_pool(name="ps_acc", bufs=1, space="PSUM") as ps_acc,
            tc.tile_pool(name="ps_bc", bufs=1, space="PSUM") as ps_bc,
        ):
            b1s = constp.tile([P, NJ], f32)
            b2s = constp.tile([P, NK], f32)
            lgs = constp.tile([P, NJ], f32)
            lbs = constp.tile([P, NJ], f32)
            w1sh = constp.tile([P, DP, 2, 32], f8)
            w1sl = constp.tile([P, DP, 2, 32], f8)
            sb1h = constp.tile([1, 1], f32)

            def emit_const_dmas():
                nc.sync.dma_start(b1s[:], b1[:])
                nc.sync.dma_start(lgs[:], lg[:])
                nc.sync.dma_start(lbs[:], lb[:])
                nc.sync.dma_start(w1sh[:], W1SH[:])
                nc.sync.dma_start(w1sl[:], W1SL[:])
                nc.sync.dma_start(sb1h[:], SB1H[:])

            ones_q = constp.tile([P, 2, 32], f8)   # lhsT for Q paired DR sums
            nc.any.memset(ones_q[:], 1.0)
            oner_f = constp.tile([1, P], f32)
            nc.any.memset(oner_f[:], 1.0)
            oner_c = constp.tile([1, P], f32r)     # lhsT for partition-broadcasts
            nc.vector.tensor_copy(oner_c[:], oner_f[:])
            eps_t = constp.tile([1, 1], f32)
            nc.any.memset(eps_t[:], LN_EPS)

            # fp8 weights, SBUF-resident for the whole kernel, streamed in
            # contiguous per-chunk DMAs staged around the first two tiles.
            w1h = wp.tile([P, NJ, DP, 2, P], f8)
            w1l = wp.tile([P, NJ, DP, 2, P], f8)
            w2h = wp.tile([P, NK, KP, 2, P], f8)
            w2l = wp.tile([P, NK, KP, 2, P], f8)

            w_cursor = [0]

            def emit_w(n):
                # next n weight chunk-pairs: W1 j-chunks, then W2 k-chunks,
                # then B2B. Each chunk is contiguous per partition.
                for _ in range(n):
                    c = w_cursor[0]
                    w_cursor[0] += 1
                    if c < NJ:
                        nc.sync.dma_start(w1h[:, c], W1H[:, c])
                        nc.sync.dma_start(w1l[:, c], W1L[:, c])
                    elif c < 2 * NJ:
                        k = c - NJ
                        nc.sync.dma_start(w2h[:, k], W2H[:, k])
                        nc.sync.dma_start(w2l[:, k], W2L[:, k])
                    elif c == 2 * NJ:
                        nc.sync.dma_start(b2s[:], b2[:])

            def emit_mm2(h_hi, h_lo, t0, tt, mid=None, post=None):
                # 3-pass compensated fp8 mm2 + b2-fold, evict on DVE.
                # mid() runs after chain 5 (the next tile's stats-broadcast
                # matmuls); post(j) runs twice per chain from chain 6 (the
                # next tile's normalize/GELU work).
                step = 0
                for k in range(NK):
                    pm = ps_mm.tile([P, TT], f32, tag="mm", name="mm2")[:, :tt]
                    for pi, (wt, ht) in enumerate(
                        ((w2h, h_hi), (w2h, h_lo), (w2l, h_hi))
                    ):
                        for kp in range(KP):
                            nc.tensor.matmul(
                                pm[:],
                                wt[:, k, kp, :, :],
                                ht[:, kp, :, :tt],
                                start=(pi == 0 and kp == 0),
                                stop=(pi == 2 and kp == KP - 1),
                                perf_mode=DR,
                            )
                    ot = op.tile([P, tt], bf16, tag=f"out{tt}", name="out", bufs=4)
                    nc.vector.tensor_scalar(
                        ot[:], pm[:], 1.0 / SW2, b2s[:, k : k + 1], Mul, Add
                    )
                    nc.sync.dma_start(outT[k, :, t0 : t0 + tt], ot[:])
                    if k == 5 and mid is not None:
                        mid()
                    if k >= 6 and post is not None:
                        for _ in range(2):
                            if step < NJ:
                                post(step)
                                step += 1
                while post is not None and step < NJ:
                    post(step)
                    step += 1

            prev = None
            backlog = []
            for tile_i, (t0, tt) in enumerate(_t_tiles(C)):
                if tile_i == 0:
                    emit_w(1)  # W1 j=0 ahead of x so the first chain starts fast
                xh = xp.tile([P, DP, 2, TT], f8, tag="xh", name="xh")
                xl = xp.tile([P, DP, 2, TT], f8, tag="xl", name="xl")
                nc.sync.dma_start(xh[:, :, :, :tt], XH[:, :, :, t0 : t0 + tt])
                nc.sync.dma_start(xl[:, :, :, :tt], XL[:, :, :, t0 : t0 + tt])
                if tile_i == 0:
                    emit_w(3)  # W1 j=1..3
                    emit_const_dmas()
                h = hp.tile(
                    [P, NJ, tt], bf16, tag=f"h{tt}", name="h",
                    bufs=(2 if tt == TT else 1),
                )
                h_hi = hxp.tile(
                    [P, KP, 2, tt], f8, tag=f"hh{tt}", name="h_hi",
                    bufs=(2 if tt == TT else 1),
                )
                h_lo = hxp.tile(
                    [P, KP, 2, tt], f8, tag=f"hl{tt}", name="h_lo",
                    bufs=(2 if tt == TT else 1),
                )
                s_ps = ps_acc.tile([32, TT], f32, tag="sacc", name="sacc")[:, :tt]
                q_ps = ps_acc.tile([32, TT], f32, tag="qacc", name="qacc")[:, :tt]

                # ---- mm1 (3-pass fp8 DR); Q ones-matmuls deferred one pair
                # so the PE never waits on the ACT evict / DVE square chain;
                # tile0's norm/GELU backlog drip-fed through tile1's loop ----
                pend_q = None
                sq = None
                for j in range(NJ):
                    if tile_i == 0 and j < 12:
                        emit_w(1)  # W1 j=4..15
                    elif tile_i == 0 and j >= 12:
                        emit_w(2)  # W2 k=0..7 behind W1
                    elif tile_i == 1:
                        emit_w(1)  # W2 k=8..15 + B2B
                    pm = ps_mm.tile([P, TT], f32, tag="mm", name="mm1")[:, :tt]
                    for pi, (wt, xt) in enumerate(((w1h, xh), (w1h, xl), (w1l, xh))):
                        for dp in range(DP):
                            nc.tensor.matmul(
                                pm[:],
                                wt[:, j, dp, :, :],
                                xt[:, dp, :, :tt],
                                start=(pi == 0 and dp == 0),
                                stop=(pi == 2 and dp == DP - 1),
                                perf_mode=DR,
                            )
                    nc.scalar.activation(
                        h[:, j, :], pm[:], Ident,
                        bias=b1s[:, j : j + 1], scale=1.0 / (SX * SW1),
                    )
                    if j % 2 == 0:
                        sq = sqp.tile([P, 2, TT], f8, tag="sq", name="sq")
                    nc.vector.tensor_mul(sq[:, j % 2, :tt], h[:, j, :], h[:, j, :])
                    if j % 2 == 1:
                        if pend_q is not None:
                            jp, sqt = pend_q
                            nc.tensor.matmul(
                                q_ps[:], ones_q[:], sqt[:, :, :tt],
                                start=(jp == 0), stop=(jp == NJ // 2 - 1),
                                perf_mode=DR,
                            )
                        pend_q = (j // 2, sq)
                    if backlog:
                        backlog.pop(0)()
                while backlog:  # finish tile0's backlog before mm2(0) reads h
                    backlog.pop(0)()
                # S-fold: the W1-column-sum row, 3-pass DR into s_ps
                for pi, (wt, xt) in enumerate(((w1sh, xh), (w1sh, xl), (w1sl, xh))):
                    for dp in range(DP):
                        nc.tensor.matmul(
                            s_ps[:],
                            wt[:, dp, :, :],
                            xt[:, dp, :, :tt],
                            start=(pi == 0 and dp == 0),
                            stop=(pi == 2 and dp == DP - 1),
                            perf_mode=DR,
                        )
                jp, sqt = pend_q
                nc.tensor.matmul(
                    q_ps[:], ones_q[:], sqt[:, :, :tt],
                    start=(jp == 0), stop=(jp == NJ // 2 - 1),
                    perf_mode=DR,
                )

                # ---- LN stats (DVE/ACT only; broadcasts happen mid-mm2) ----
                mu = statp.tile([1, TT], f32, tag="mu", name="mu")[:, :tt]
                nc.vector.tensor_scalar(
                    mu[:], s_ps[0:1, :], 1.0 / (SX * SW1S * H), sb1h[:], Mul, Add
                )
                tmp = statp.tile([1, TT], f32, tag="tmp", name="tmp")[:, :tt]
                nc.vector.tensor_scalar_mul(tmp[:], q_ps[0:1, :], 1.0 / H)
                tmp2 = statp.tile([1, TT], f32, tag="tmp2", name="tmp2")[:, :tt]
                nc.vector.tensor_mul(tmp2[:], mu[:], mu[:])
                nc.vector.tensor_sub(tmp[:], tmp[:], tmp2[:])          # var
                nc.scalar.activation(tmp2[:], tmp[:], Sqrt, bias=eps_t[:])  # std
                nc.vector.reciprocal(tmp[:], tmp2[:])                  # rstd
                a_row = statp.tile([1, TT], f32r, tag="a_row", name="a_row")[:, :tt]
                nc.vector.tensor_copy(a_row[:], tmp[:])
                b_row = statp.tile([1, TT], f32r, tag="b_row", name="b_row")[:, :tt]
                nc.vector.tensor_mul(b_row[:], mu[:], tmp[:])

                a_sb = statp.tile([P, TT], bf16, tag="a_sb", name="a_sb", bufs=2)
                b_sb = statp.tile([P, TT], bf16, tag="b_sb", name="b_sb", bufs=2)

                def emit_bc(a_row=a_row, b_row=b_row, a_sb=a_sb, b_sb=b_sb, tt=tt):
                    a_bc = ps_bc.tile([P, TT], f32, tag="a_bc", name="a_bc")[:, :tt]
                    nc.tensor.matmul(
                        a_bc[:], oner_c[:], a_row[:], start=True, stop=True
                    )
                    b_bc = ps_bc.tile([P, TT], f32, tag="b_bc", name="b_bc")[:, :tt]
                    nc.tensor.matmul(
                        b_bc[:], oner_c[:], b_row[:], start=True, stop=True
                    )
                    nc.vector.tensor_copy(a_sb[:, :tt], a_bc[:])
                    nc.vector.tensor_copy(b_sb[:, :tt], b_bc[:])

                def emit_norm_gelu(
                    j, h=h, h_hi=h_hi, h_lo=h_lo, a_sb=a_sb, b_sb=b_sb, tt=tt
                ):
                    # normalize (DVE bf16) + GELU twice (ACT: fp8 h_hi, bf16
                    # in place) + h_lo residual (DVE)
                    jp, pl = j // 2, j % 2
                    hj = h[:, j, :tt]
                    nc.vector.tensor_mul(hj, hj, a_sb[:, :tt])
                    nc.vector.tensor_sub(hj, hj, b_sb[:, :tt])
                    nc.scalar.activation(
                        h_hi[:, jp, pl, :tt], hj, Gelu,
                        bias=lbs[:, j : j + 1], scale=lgs[:, j : j + 1],
                    )
                    nc.scalar.activation(
                        hj, hj, Gelu, bias=lbs[:, j : j + 1], scale=lgs[:, j : j + 1]
                    )
                    nc.vector.tensor_sub(
                        h_lo[:, jp, pl, :tt], hj, h_hi[:, jp, pl, :tt]
                    )

                # ---- previous tile's mm2 on the PE, with this tile's
                # broadcasts at chain 5 and norm/GELU from chain 6 ----
                if prev is not None:
                    emit_mm2(*prev, mid=emit_bc, post=emit_norm_gelu)
                else:
                    backlog.append(emit_bc)
                    backlog.extend(
                        (lambda j=j, f=emit_norm_gelu: f(j)) for j in range(NJ)
                    )
                prev = (h_hi, h_lo, t0, tt)

            emit_w(2 * NJ + 1 - w_cursor[0])
            for fn in backlog:
                fn()
            emit_mm2(*prev)

    nc.compile()
    return nc


def _route(x64, Wg64, bg64):
    """Host gating: per-token top-2 expert ids and renormalized weights."""
    logits = x64 @ Wg64 + bg64                      # [N, E] fp64
    order = np.argsort(-logits, axis=1, kind="stable")[:, :TOPK]
    l0 = np.take_along_axis(logits, order, axis=1)  # [N, 2] descending
    w0 = 1.0 / (1.0 + np.exp(l0[:, 1] - l0[:, 0]))
    w = np.stack([w0, 1.0 - w0], axis=1)
    return order, w


def _split8(a):
    hi = a.astype(F8)
    lo = (a - hi.astype(np.float32)).astype(F8)
    return hi, lo


def kernel(x, W1, b1, ln_g, ln_b, W2, b2, Wg, bg):
    x = np.ascontiguousarray(np.asarray(x, dtype=np.float32))
    W1 = np.asarray(W1, dtype=np.float32)
    b1 = np.asarray(b1, dtype=np.float32)
    ln_g = np.asarray(ln_g, dtype=np.float32)
    ln_b = np.asarray(ln_b, dtype=np.float32)
    W2 = np.asarray(W2, dtype=np.float32)
    b2 = np.asarray(b2, dtype=np.float32)
    Wg = np.asarray(Wg, dtype=np.float32)
    bg = np.asarray(bg, dtype=np.float32)
    N = x.shape[0]

    order, w = _route(x.astype(np.float64), Wg.astype(np.float64), bg.astype(np.float64))

    tok_idx, tok_w = [], []
    for e in range(E):
        sel = np.nonzero((order[:, 0] == e) | (order[:, 1] == e))[0]
        we = np.where(order[sel, 0] == e, w[sel, 0], w[sel, 1]).astype(np.float32)
        tok_idx.append(sel)
        tok_w.append(we)
    C = max(GRAN, int(-(-max(len(s) for s in tok_idx) // GRAN)) * GRAN)

    if C not in _kernel_cache:
        _kernel_cache[C] = _build(C)
    nc = _kernel_cache[C]

    in_maps = []
    for e in range(E):
        idx = np.zeros(C, dtype=np.int64)
        idx[: len(tok_idx[e])] = tok_idx[e]
        xg = x[idx] * SX                              # [C, D]
        xh, xl = _split8(xg)
        # [C, D] -> [P, DP, 2, C]
        xh_d = np.ascontiguousarray(xh.reshape(C, DP, 2, P).transpose(3, 1, 2, 0))
        xl_d = np.ascontiguousarray(xl.reshape(C, DP, 2, P).transpose(3, 1, 2, 0))
        w1h, w1l = _split8(W1[e] * SW1)               # [D, H]
        w1h_d = np.ascontiguousarray(
            w1h.reshape(DP, 2, P, NJ, P).transpose(2, 3, 0, 1, 4)
        )
        w1l_d = np.ascontiguousarray(
            w1l.reshape(DP, 2, P, NJ, P).transpose(2, 3, 0, 1, 4)
        )
        # S-fold: column-sum of W1 (scaled), replicated over 32 lhsT columns
        w1s = W1[e].sum(axis=1) * SW1S                # [D]
        w1sh, w1sl = _split8(w1s)
        w1sh_d = np.ascontiguousarray(np.broadcast_to(
            w1sh.reshape(DP, 2, P).transpose(2, 0, 1)[:, :, :, None], (P, DP, 2, 32)
        ).astype(F8))
        w1sl_d = np.ascontiguousarray(np.broadcast_to(
            w1sl.reshape(DP, 2, P).transpose(2, 0, 1)[:, :, :, None], (P, DP, 2, 32)
        ).astype(F8))
        sb1h_d = np.full((1, 1), b1[e].sum() / H, dtype=np.float32)
        w2h, w2l = _split8(W2[e] * SW2)               # [H, H]
        w2h_d = np.ascontiguousarray(
            w2h.reshape(KP, 2, P, NK, P).transpose(2, 3, 0, 1, 4)
        )
        w2l_d = np.ascontiguousarray(
            w2l.reshape(KP, 2, P, NK, P).transpose(2, 3, 0, 1, 4)
        )
        in_maps.append(
            {
                "XH": xh_d,
                "XL": xl_d,
                "W1H": w1h_d,
                "W1L": w1l_d,
                "W1SH": w1sh_d,
                "W1SL": w1sl_d,
                "SB1H": sb1h_d,
                "W2H": w2h_d,
                "W2L": w2l_d,
                "b1": np.ascontiguousarray(b1[e].reshape(NJ, P).T),
                "b2": np.ascontiguousarray(b2[e].reshape(NK, P).T),
                "lg": np.ascontiguousarray(ln_g[e].reshape(NJ, P).T),
                "lb": np.ascontiguousarray(ln_b[e].reshape(NJ, P).T),
            }
        )

    results = _run(C, nc, in_maps)

    y = np.zeros((N, H), dtype=np.float32)
    for e in range(E):
        cnt = len(tok_idx[e])
        eoT = results[e]["outT"].reshape(H, C).astype(np.float32)
        y[tok_idx[e]] += tok_w[e][:, None] * eoT[:, :cnt].T
    return y


_neff_cache: dict[int, str] = {}


def _run(C, nc, in_maps):
    if axon_active():
        # PJRT path; NEFF compile is cached by libneuronxla.
        return run_bass_kernel_spmd(nc, in_maps, core_ids=list(range(E))).results
    # Native path: compile once per capacity, then execute the cached NEFF.
    from concourse.bass_utils import compile_bass_kernel, run_neff

    if C not in _neff_cache:
        _neff_cache[C] = compile_bass_kernel(nc, tempfile.mkdtemp())
    out_maps = [{"outT": np.zeros((NK, P, C), dtype=BF)} for _ in range(E)]
    in_maps = [m.copy() for m in in_maps]
    if nc.partition_id_tensor:
        for core_id, m in enumerate(in_maps):
            m[nc.partition_id_tensor.name] = np.array([[core_id]], dtype=np.uint32)
    return run_neff(
        _neff_cache[C],
        in_maps,
        out_maps,
        core_ids=list(range(E)),
        has_collectives=False,
    )


# revision 11
# speedup vs baseline: 1.7548x; 1.0060x over previous
# MoE (top-2 of 8 experts) kernel for 8 Trainium2 NeuronCores.
#
# Strategy: expert-parallel sparse routing with fp8 DoubleRow matmuls.
# Host computes the gating network and per-expert token lists; core e runs
# expert e's FFN (x@W1+b1 -> LayerNorm -> erf-GELU -> @W2+b2) on its routed
# tokens. Both matmuls run as fp8(e4m3) DoubleRow (2 k-planes per
# instruction, 0.5 cyc/row = 4x the f32r MAC rate) with hi/lo error
# compensation: A@B ~= Ah@Bh + Al@Bh + Ah@Bl where Ah=fp8(A), Al=fp8(A-Ah).
# All weights live in SBUF (12MB fp8), loaded once in contiguous per-chunk
# DMAs. LayerNorm S-sums are folded into mm1 via a W1-column-sum lhsT row;
# Q-sums use a paired-fp8 DoubleRow ones-matmul on h^2; per-token stats
# broadcast across partitions with K=1 f32r matmuls, emitted mid-mm2 so the
# PE never waits on the stats chain. b2 is folded into the mm2 PSUM chain
# via a constant fp8 matmul so the mm2 evict is a single DVE
# tensor_scalar_mul (keeps ACT free for the GELUs). The first tile's
# normalize/GELU backlog is drip-fed through the second tile's mm1 loop.

import tempfile

import ml_dtypes
import numpy as np

import concourse.bacc as bacc
import concourse.mybir as mybir
import concourse.tile as tile
from concourse._compat import axon_active
from concourse.bass_utils import run_bass_kernel_spmd

P = 128
D, H, E, TOPK = 1024, 2048, 8, 2
DP, KP, NJ, NK = D // 256, H // 256, H // P, H // P  # 4, 8, 16, 16
LN_EPS = 1e-5
TT = 512           # main token tile
GRAN = 16          # capacity granularity
SX, SW1, SW2 = 16.0, 256.0, 256.0   # fp8 pre-quantization scales
SW1S = 32.0        # scale for the W1 column-sum row (S-fold)
F8 = ml_dtypes.float8_e4m3
BF = ml_dtypes.bfloat16

_kernel_cache: dict[int, object] = {}


def _t_tiles(C):
    tiles, t0 = [], 0
    while t0 < C:
        tt = TT if C - t0 >= TT else C - t0
        tiles.append((t0, tt))
        t0 += tt
    # Tail tile last: its cheap mm2 is the only un-overlapped one, and
    # full-size norm/GELU phases pair with full-size mm2 phases.
    return tiles


def _build(C: int):
    f32, f32r, bf16, f8 = (
        mybir.dt.float32, mybir.dt.float32r, mybir.dt.bfloat16, mybir.dt.float8e4
    )
    DR = mybir.MatmulPerfMode.DoubleRow
    Mul, Add = mybir.AluOpType.mult, mybir.AluOpType.add
    nc = bacc.Bacc("TRN2", target_bir_lowering=False, debug=False, num_devices=8)
    XH = nc.dram_tensor("XH", [P, DP, 2, C], f8, kind="ExternalInput").ap()
    XL = nc.dram_tensor("XL", [P, DP, 2, C], f8, kind="ExternalInput").ap()
    W1H = nc.dram_tensor("W1H", [P, NJ, DP, 2, P], f8, kind="ExternalInput").ap()
    W1L = nc.dram_tensor("W1L", [P, NJ, DP, 2, P], f8, kind="ExternalInput").ap()
    W1SH = nc.dram_tensor("W1SH", [P, DP, 2, 32], f8, kind="ExternalInput").ap()
    W1SL = nc.dram_tensor("W1SL", [P, DP, 2, 32], f8, kind="ExternalInput").ap()
    SB1H = nc.dram_tensor("SB1H", [1, 1], f32, kind="ExternalInput").ap()
    W2H = nc.dram_tensor("W2H", [P, NK, KP, 2, P], f8, kind="ExternalInput").ap()
    W2L = nc.dram_tensor("W2L", [P, NK, KP, 2, P], f8, kind="ExternalInput").ap()
    b1 = nc.dram_tensor("b1", [P, NJ], f32, kind="ExternalInput").ap()
    b2 = nc.dram_tensor("b2", [P, NK], f32, kind="ExternalInput").ap()
    lg = nc.dram_tensor("lg", [P, NJ], f32, kind="ExternalInput").ap()
    lb = nc.dram_tensor("lb", [P, NJ], f32, kind="ExternalInput").ap()
    outT = nc.dram_tensor("outT", [NK, P, C], bf16, kind="ExternalOutput").ap()

    Gelu = mybir.ActivationFunctionType.Gelu
    Sqrt = mybir.ActivationFunctionType.Sqrt
    Ident = mybir.ActivationFunctionType.Identity

    with tile.TileContext(nc) as tc:
        with (
            tc.tile_pool(name="const", bufs=1) as constp,
            tc.tile_pool(name="wp", bufs=1) as wp,
            tc.tile_pool(name="xp", bufs=1) as xp,
            tc.tile_pool(name="hp", bufs=2) as hp,
            tc.tile_pool(name="hxp", bufs=2) as hxp,
            tc.tile_pool(name="sqp", bufs=2) as sqp,
            tc.tile_pool(name="op", bufs=2) as op,
            tc.tile_pool(name="statp", bufs=1) as statp,
            tc.tile_pool(name="ps_mm", bufs=4, space="PSUM") as ps_mm,
            tc.tile_pool(name="ps_acc", bufs=1, space="PSUM") as ps_acc,
            tc.tile_pool(name="ps_bc", bufs=1, space="PSUM") as ps_bc,
        ):
            b1s = constp.tile([P, NJ], f32)
            b2s = constp.tile([P, NK], f32)
            lgs = constp.tile([P, NJ], f32)
            lbs = constp.tile([P, NJ], f32)
            w1sh = constp.tile([P, DP, 2, 32], f8)
            w1sl = constp.tile([P, DP, 2, 32], f8)
            sb1h = constp.tile([1, 1], f32)

            def emit_const_dmas():
                nc.sync.dma_start(b1s[:], b1[:])
                nc.sync.dma_start(lgs[:], lg[:])
                nc.sync.dma_start(lbs[:], lb[:])
                nc.sync.dma_start(w1sh[:], W1SH[:])
                nc.sync.dma_start(w1sl[:], W1SL[:])
                nc.sync.dma_start(sb1h[:], SB1H[:])

            ones_q = constp.tile([P, 2, 32], f8)   # lhsT for Q paired DR sums
            nc.any.memset(ones_q[:], 1.0)
            oner_f = constp.tile([1, P], f32)
            nc.any.memset(oner_f[:], 1.0)
            oner_c = constp.tile([1, P], f32r)     # lhsT for partition-broadcasts
            nc.vector.tensor_copy(oner_c[:], oner_f[:])
            eps_t = constp.tile([1, 1], f32)
            nc.any.memset(eps_t[:], LN_EPS)

            # fp8 weights, SBUF-resident for the whole kernel, streamed in
            # contiguous per-chunk DMAs staged around the first two tiles.
            w1h = wp.tile([P, NJ, DP, 2, P], f8)
            w1l = wp.tile([P, NJ, DP, 2, P], f8)
            w2h = wp.tile([P, NK, KP, 2, P], f8)
            w2l = wp.tile([P, NK, KP, 2, P], f8)

            w_cursor = [0]

            def emit_w(n):
                # next n weight chunk-pairs: W1 j-chunks, then W2 k-chunks,
                # then B2B. Each chunk is contiguous per partition.
                for _ in range(n):
                    c = w_cursor[0]
                    w_cursor[0] += 1
                    if c < NJ:
                        nc.sync.dma_start(w1h[:, c], W1H[:, c])
                        nc.sync.dma_start(w1l[:, c], W1L[:, c])
                    elif c < 2 * NJ:
                        k = c - NJ
                        nc.sync.dma_start(w2h[:, k], W2H[:, k])
                        nc.sync.dma_start(w2l[:, k], W2L[:, k])
                    elif c == 2 * NJ:
                        nc.sync.dma_start(b2s[:], b2[:])

            def emit_mm2(h_hi, h_lo, t0, tt, mid=None, post=None):
                # 3-pass compensated fp8 mm2 + b2-fold, evict on DVE.
                # mid() runs after chain 5 (the next tile's stats-broadcast
                # matmuls); post(j) runs twice per chain from chain 6 (the
                # next tile's normalize/GELU work).
                step = 0
                for k in range(NK):
                    pm = ps_mm.tile([P, TT], f32, tag="mm", name="mm2")[:, :tt]
                    for pi, (wt, ht) in enumerate(
                        ((w2h, h_hi), (w2h, h_lo), (w2l, h_hi))
                    ):
                        for kp in range(KP):
                            nc.tensor.matmul(
                                pm[:],
                                wt[:, k, kp, :, :],
                                ht[:, kp, :, :tt],
                                start=(pi == 0 and kp == 0),
                                stop=(pi == 2 and kp == KP - 1),
                                perf_mode=DR,
                            )
                    ot = op.tile([P, tt], bf16, tag=f"out{tt}", name="out", bufs=4)
                    nc.vector.tensor_scalar(
                        ot[:], pm[:], 1.0 / SW2, b2s[:, k : k + 1], Mul, Add
                    )
                    nc.sync.dma_start(outT[k, :, t0 : t0 + tt], ot[:])
                    if k == 5 and mid is not None:
                        mid()
                    if k >= 6 and post is not None:
                        for _ in range(2):
                            if step < NJ:
                                post(step)
                                step += 1
                while post is not None and step < NJ:
                    post(step)
                    step += 1

            prev = None
            backlog = []
            for tile_i, (t0, tt) in enumerate(_t_tiles(C)):
                if tile_i == 0:
                    emit_w(1)  # W1 j=0 ahead of x so the first chain starts fast
                xh = xp.tile([P, DP, 2, TT], f8, tag="xh", name="xh")
                xl = xp.tile([P, DP, 2, TT], f8, tag="xl", name="xl")
                nc.sync.dma_start(xh[:, :, :, :tt], XH[:, :, :, t0 : t0 + tt])
                nc.sync.dma_start(xl[:, :, :, :tt], XL[:, :, :, t0 : t0 + tt])
                if tile_i == 0:
                    emit_w(3)  # W1 j=1..3
                    emit_const_dmas()
                h = hp.tile(
                    [P, NJ, tt], bf16, tag=f"h{tt}", name="h",
                    bufs=(2 if tt == TT else 1),
                )
                h_hi = hxp.tile(
                    [P, KP, 2, tt], f8, tag=f"hh{tt}", name="h_hi",
                    bufs=(2 if tt == TT else 1),
                )
                h_lo = hxp.tile(
                    [P, KP, 2, tt], f8, tag=f"hl{tt}", name="h_lo",
                    bufs=(2 if tt == TT else 1),
                )
                s_ps = ps_acc.tile([32, TT], f32, tag="sacc", name="sacc")[:, :tt]
                q_ps = ps_acc.tile([32, TT], f32, tag="qacc", name="qacc")[:, :tt]

                # ---- mm1 (3-pass fp8 DR); Q ones-matmuls deferred one pair
                # so the PE never waits on the ACT evict / DVE square chain;
                # tile0's norm/GELU backlog drip-fed through tile1's loop ----
                pend_q = None
                sq = None
                for j in range(NJ):
                    if tile_i == 0 and j < 12:
                        emit_w(1)  # W1 j=4..15
                    elif tile_i == 0 and j >= 12:
                        emit_w(2)  # W2 k=0..7 behind W1
                    elif tile_i == 1:
                        emit_w(1)  # W2 k=8..15 + B2B
                    pm = ps_mm.tile([P, TT], f32, tag="mm", name="mm1")[:, :tt]
                    for pi, (wt, xt) in enumerate(((w1h, xh), (w1h, xl), (w1l, xh))):
                        for dp in range(DP):
                            nc.tensor.matmul(
                                pm[:],
                                wt[:, j, dp, :, :],
                                xt[:, dp, :, :tt],
                                start=(pi == 0 and dp == 0),
                                stop=(pi == 2 and dp == DP - 1),
                                perf_mode=DR,
                            )
                    nc.scalar.activation(
                        h[:, j, :], pm[:], Ident,
                        bias=b1s[:, j : j + 1], scale=1.0 / (SX * SW1),
                    )
                    if j % 2 == 0:
                        sq = sqp.tile([P, 2, TT], f8, tag="sq", name="sq")
                    nc.vector.tensor_mul(sq[:, j % 2, :tt], h[:, j, :], h[:, j, :])
                    if j % 2 == 1:
                        if pend_q is not None:
                            jp, sqt = pend_q
                            nc.tensor.matmul(
                                q_ps[:], ones_q[:], sqt[:, :, :tt],
                                start=(jp == 0), stop=(jp == NJ // 2 - 1),
                                perf_mode=DR,
                            )
                        pend_q = (j // 2, sq)
                    if backlog:
                        backlog.pop(0)()
                while backlog:  # finish tile0's backlog before mm2(0) reads h
                    backlog.pop(0)()
                # S-fold: the W1-column-sum row, 3-pass DR into s_ps
                for pi, (wt, xt) in enumerate(((w1sh, xh), (w1sh, xl), (w1sl, xh))):
                    for dp in range(DP):
                        nc.tensor.matmul(
                            s_ps[:],
                            wt[:, dp, :, :],
                            xt[:, dp, :, :tt],
                            start=(pi == 0 and dp == 0),
                            stop=(pi == 2 and dp == DP - 1),
                            perf_mode=DR,
                        )
                jp, sqt = pend_q
                nc.tensor.matmul(
                    q_ps[:], ones_q[:], sqt[:, :, :tt],
                    start=(jp == 0), stop=(jp == NJ // 2 - 1),
                    perf_mode=DR,
                )

                # ---- LN stats (DVE/ACT only; broadcasts happen mid-mm2) ----
                mu = statp.tile([1, TT], f32, tag="mu", name="mu")[:, :tt]
                nc.vector.tensor_scalar(
                    mu[:], s_ps[0:1, :], 1.0 / (SX * SW1S * H), sb1h[:], Mul, Add
                )
                tmp = statp.tile([1, TT], f32, tag="tmp", name="tmp")[:, :tt]
                nc.vector.tensor_scalar_mul(tmp[:], q_ps[0:1, :], 1.0 / H)
                tmp2 = statp.tile([1, TT], f32, tag="tmp2", name="tmp2")[:, :tt]
                nc.vector.tensor_mul(tmp2[:], mu[:], mu[:])
                nc.vector.tensor_sub(tmp[:], tmp[:], tmp2[:])          # var
                nc.scalar.activation(tmp2[:], tmp[:], Sqrt, bias=eps_t[:])  # std
                nc.vector.reciprocal(tmp[:], tmp2[:])                  # rstd
                a_row = statp.tile([1, TT], f32r, tag="a_row", name="a_row")[:, :tt]
                nc.vector.tensor_copy(a_row[:], tmp[:])
                b_row = statp.tile([1, TT], f32r, tag="b_row", name="b_row")[:, :tt]
                nc.vector.tensor_mul(b_row[:], mu[:], tmp[:])

                a_sb = statp.tile([P, TT], bf16, tag="a_sb", name="a_sb", bufs=2)
                b_sb = statp.tile([P, TT], bf16, tag="b_sb", name="b_sb", bufs=2)

                def emit_bc(a_row=a_row, b_row=b_row, a_sb=a_sb, b_sb=b_sb, tt=tt):
                    a_bc = ps_bc.tile([P, TT], f32, tag="a_bc", name="a_bc")[:, :tt]
                    nc.tensor.matmul(
                        a_bc[:], oner_c[:], a_row[:], start=True, stop=True
                    )
                    b_bc = ps_bc.tile([P, TT], f32, tag="b_bc", name="b_bc")[:, :tt]
                    nc.tensor.matmul(
                        b_bc[:], oner_c[:], b_row[:], start=True, stop=True
                    )
                    nc.vector.tensor_copy(a_sb[:, :tt], a_bc[:])
                    nc.vector.tensor_copy(b_sb[:, :tt], b_bc[:])

                def emit_norm_gelu(
                    j, h=h, h_hi=h_hi, h_lo=h_lo, a_sb=a_sb, b_sb=b_sb, tt=tt
                ):
                    # normalize (DVE bf16) + GELU twice (ACT: fp8 h_hi, bf16
                    # in place) + h_lo residual (DVE)
                    jp, pl = j // 2, j % 2
                    hj = h[:, j, :tt]
                    nc.vector.tensor_mul(hj, hj, a_sb[:, :tt])
                    nc.vector.tensor_sub(hj, hj, b_sb[:, :tt])
                    nc.scalar.activation(
                        h_hi[:, jp, pl, :tt], hj, Gelu,
                        bias=lbs[:, j : j + 1], scale=lgs[:, j : j + 1],
                    )
                    nc.scalar.activation(
                        hj, hj, Gelu, bias=lbs[:, j : j + 1], scale=lgs[:, j : j + 1]
                    )
                    nc.vector.tensor_sub(
                        h_lo[:, jp, pl, :tt], hj, h_hi[:, jp, pl, :tt]
                    )

                # ---- previous tile's mm2 on the PE, with this tile's
                # broadcasts at chain 5 and norm/GELU from chain 6 ----
                if prev is not None:
                    emit_mm2(*prev, mid=emit_bc, post=emit_norm_gelu)
                else:
                    backlog.append(emit_bc)
                    backlog.extend(
                        (lambda j=j, f=emit_norm_gelu: f(j)) for j in range(NJ)
                    )
                prev = (h_hi, h_lo, t0, tt)

            emit_w(2 * NJ + 1 - w_cursor[0])
            for fn in backlog:
                fn()
            emit_mm2(*prev)

    nc.compile()
    return nc


def _route(x64, Wg64, bg64):
    """Host gating: per-token top-2 expert ids and renormalized weights."""
    logits = x64 @ Wg64 + bg64                      # [N, E] fp64
    order = np.argsort(-logits, axis=1, kind="stable")[:, :TOPK]
    l0 = np.take_along_axis(logits, order, axis=1)  # [N, 2] descending
    w0 = 1.0 / (1.0 + np.exp(l0[:, 1] - l0[:, 0]))
    w = np.stack([w0, 1.0 - w0], axis=1)
    return order, w


def _split8(a):
    hi = a.astype(F8)
    lo = (a - hi.astype(np.float32)).astype(F8)
    return hi, lo


def kernel(x, W1, b1, ln_g, ln_b, W2, b2, Wg, bg):
    x = np.ascontiguousarray(np.asarray(x, dtype=np.float32))
    W1 = np.asarray(W1, dtype=np.float32)
    b1 = np.asarray(b1, dtype=np.float32)
    ln_g = np.asarray(ln_g, dtype=np.float32)
    ln_b = np.asarray(ln_b, dtype=np.float32)
    W2 = np.asarray(W2, dtype=np.float32)
    b2 = np.asarray(b2, dtype=np.float32)
    Wg = np.asarray(Wg, dtype=np.float32)
    bg = np.asarray(bg, dtype=np.float32)
    N = x.shape[0]

    order, w = _route(x.astype(np.float64), Wg.astype(np.float64), bg.astype(np.float64))

    tok_idx, tok_w = [], []
    for e in range(E):
        sel = np.nonzero((order[:, 0] == e) | (order[:, 1] == e))[0]
        we = np.where(order[sel, 0] == e, w[sel, 0], w[sel, 1]).astype(np.float32)
        tok_idx.append(sel)
        tok_w.append(we)
    C = max(GRAN, int(-(-max(len(s) for s in tok_idx) // GRAN)) * GRAN)

    if C not in _kernel_cache:
        _kernel_cache[C] = _build(C)
    nc = _kernel_cache[C]

    in_maps = []
    for e in range(E):
        idx = np.zeros(C, dtype=np.int64)
        idx[: len(tok_idx[e])] = tok_idx[e]
        xg = x[idx] * SX                              # [C, D]
        xh, xl = _split8(xg)
        # [C, D] -> [P, DP, 2, C]
        xh_d = np.ascontiguousarray(xh.reshape(C, DP, 2, P).transpose(3, 1, 2, 0))
        xl_d = np.ascontiguousarray(xl.reshape(C, DP, 2, P).transpose(3, 1, 2, 0))
        w1h, w1l = _split8(W1[e] * SW1)               # [D, H]
        w1h_d = np.ascontiguousarray(
            w1h.reshape(DP, 2, P, NJ, P).transpose(2, 3, 0, 1, 4)
        )
        w1l_d = np.ascontiguousarray(
            w1l.reshape(DP, 2, P, NJ, P).transpose(2, 3, 0, 1, 4)
        )
        # S-fold: column-sum of W1 (scaled), replicated over 32 lhsT columns
        w1s = W1[e].sum(axis=1) * SW1S                # [D]
        w1sh, w1sl = _split8(w1s)
        w1sh_d = np.ascontiguousarray(np.broadcast_to(
            w1sh.reshape(DP, 2, P).transpose(2, 0, 1)[:, :, :, None], (P, DP, 2, 32)
        ).astype(F8))
        w1sl_d = np.ascontiguousarray(np.broadcast_to(
            w1sl.reshape(DP, 2, P).transpose(2, 0, 1)[:, :, :, None], (P, DP, 2, 32)
        ).astype(F8))
        sb1h_d = np.full((1, 1), b1[e].sum() / H, dtype=np.float32)
        w2h, w2l = _split8(W2[e] * SW2)               # [H, H]
        w2h_d = np.ascontiguousarray(
            w2h.reshape(KP, 2, P, NK, P).transpose(2, 3, 0, 1, 4)
        )
        w2l_d = np.ascontiguousarray(
            w2l.reshape(KP, 2, P, NK, P).transpose(2, 3, 0, 1, 4)
        )
        in_maps.append(
            {
                "XH": xh_d,
                "XL": xl_d,
                "W1H": w1h_d,
                "W1L": w1l_d,
                "W1SH": w1sh_d,
                "W1SL": w1sl_d,
                "SB1H": sb1h_d,
                "W2H": w2h_d,
                "W2L": w2l_d,
                "b1": np.ascontiguousarray(b1[e].reshape(NJ, P).T),
                "b2": np.ascontiguousarray(b2[e].reshape(NK, P).T),
                "lg": np.ascontiguousarray(ln_g[e].reshape(NJ, P).T),
                "lb": np.ascontiguousarray(ln_b[e].reshape(NJ, P).T),
            }
        )

    results = _run(C, nc, in_maps)

    y = np.zeros((N, H), dtype=np.float32)
    for e in range(E):
        cnt = len(tok_idx[e])
        eoT = results[e]["outT"].reshape(H, C).astype(np.float32)
        y[tok_idx[e]] += tok_w[e][:, None] * eoT[:, :cnt].T
    return y


_neff_cache: dict[int, str] = {}


def _run(C, nc, in_maps):
    if axon_active():
        # PJRT path; NEFF compile is cached by libneuronxla.
        return run_bass_kernel_spmd(nc, in_maps, core_ids=list(range(E))).results
    # Native path: compile once per capacity, then execute the cached NEFF.
    from concourse.bass_utils import compile_bass_kernel, run_neff

    if C not in _neff_cache:
        _neff_cache[C] = compile_bass_kernel(nc, tempfile.mkdtemp())
    out_maps = [{"outT": np.zeros((NK, P, C), dtype=BF)} for _ in range(E)]
    in_maps = [m.copy() for m in in_maps]
    if nc.partition_id_tensor:
        for core_id, m in enumerate(in_maps):
            m[nc.partition_id_tensor.name] = np.array([[core_id]], dtype=np.uint32)
    return run_neff(
        _neff_cache[C],
        in_maps,
        out_maps,
        core_ids=list(range(E)),
        has_collectives=False,
    )


# revision 15
# speedup vs baseline: 1.7835x; 1.0164x over previous
# MoE (top-2 of 8 experts) kernel for 8 Trainium2 NeuronCores.
#
# Strategy: expert-parallel sparse routing with fp8 DoubleRow matmuls.
# Host computes the gating network and per-expert token lists; core e runs
# expert e's FFN (x@W1+b1 -> LayerNorm -> erf-GELU -> @W2+b2) on its routed
# tokens. Both matmuls run as fp8(e4m3) DoubleRow (2 k-planes per
# instruction, 0.5 cyc/row = 4x the f32r MAC rate) with hi/lo error
# compensation: A@B ~= Ah@Bh + Al@Bh + Ah@Bl where Ah=fp8(A), Al=fp8(A-Ah).
# All weights live in SBUF (12MB fp8), loaded once in contiguous per-chunk
# DMAs. LayerNorm S-sums are folded into mm1 via a W1-column-sum lhsT row;
# Q-sums use a paired-fp8 DoubleRow ones-matmul on h^2; per-token stats
# broadcast across partitions with K=1 f32r matmuls, emitted mid-mm2 so the
# PE never waits on the stats chain. b2 is folded into the mm2 PSUM chain
# via a constant fp8 matmul so the mm2 evict is a single DVE
# tensor_scalar_mul (keeps ACT free for the GELUs). The first tile's
# normalize/GELU backlog is drip-fed through the second tile's mm1 loop.

import tempfile

import ml_dtypes
import numpy as np

import concourse.bacc as bacc
import concourse.mybir as mybir
import concourse.tile as tile
from concourse._compat import axon_active
from concourse.bass_utils import run_bass_kernel_spmd

P = 128
D, H, E, TOPK = 1024, 2048, 8, 2
DP, KP, NJ, NK = D // 256, H // 256, H // P, H // P  # 4, 8, 16, 16
LN_EPS = 1e-5
TT = 512           # main token tile
GRAN = 16          # capacity granularity
SX, SW1, SW2 = 16.0, 256.0, 256.0   # fp8 pre-quantization scales
SW1S = 32.0        # scale for the W1 column-sum row (S-fold)
F8 = ml_dtypes.float8_e4m3
BF = ml_dtypes.bfloat16

_kernel_cache: dict[int, object] = {}


def _t_tiles(C):
    tiles, t0 = [], 0
    while t0 < C:
        tt = TT if C - t0 >= TT else C - t0
        tiles.append((t0, tt))
        t0 += tt
    # Tail tile last: its cheap mm2 is the only un-overlapped one, and
    # full-size norm/GELU phases pair with full-size mm2 phases.
    return tiles


def _build(C: int):
    f32, f32r, bf16, f8 = (
        mybir.dt.float32, mybir.dt.float32r, mybir.dt.bfloat16, mybir.dt.float8e4
    )
    DR = mybir.MatmulPerfMode.DoubleRow
    Mul, Add = mybir.AluOpType.mult, mybir.AluOpType.add
    nc = bacc.Bacc("TRN2", target_bir_lowering=False, debug=False, num_devices=8)
    XH = nc.dram_tensor("XH", [P, DP, 2, C], f8, kind="ExternalInput").ap()
    XL = nc.dram_tensor("XL", [P, DP, 2, C], f8, kind="ExternalInput").ap()
    W1H = nc.dram_tensor("W1H", [P, NJ, DP, 2, P], f8, kind="ExternalInput").ap()
    W1L = nc.dram_tensor("W1L", [P, NJ, DP, 2, P], f8, kind="ExternalInput").ap()
    W1SH = nc.dram_tensor("W1SH", [P, DP, 2, 32], f8, kind="ExternalInput").ap()
    W1SL = nc.dram_tensor("W1SL", [P, DP, 2, 32], f8, kind="ExternalInput").ap()
    SB1H = nc.dram_tensor("SB1H", [1, 1], f32, kind="ExternalInput").ap()
    W2H = nc.dram_tensor("W2H", [P, NK, KP, 2, P], f8, kind="ExternalInput").ap()
    W2L = nc.dram_tensor("W2L", [P, NK, KP, 2, P], f8, kind="ExternalInput").ap()
    b1 = nc.dram_tensor("b1", [P, NJ], f32, kind="ExternalInput").ap()
    b2 = nc.dram_tensor("b2", [P, NK], f32, kind="ExternalInput").ap()
    lg = nc.dram_tensor("lg", [P, NJ], f32, kind="ExternalInput").ap()
    lb = nc.dram_tensor("lb", [P, NJ], f32, kind="ExternalInput").ap()
    outT = nc.dram_tensor("outT", [NK, P, C], bf16, kind="ExternalOutput").ap()

    Gelu = mybir.ActivationFunctionType.Gelu
    Sqrt = mybir.ActivationFunctionType.Sqrt
    Ident = mybir.ActivationFunctionType.Identity

    with tile.TileContext(nc) as tc:
        with (
            tc.tile_pool(name="const", bufs=1) as constp,
            tc.tile_pool(name="wp", bufs=1) as wp,
            tc.tile_pool(name="xp", bufs=1) as xp,
            tc.tile_pool(name="hp", bufs=2) as hp,
            tc.tile_pool(name="hxp", bufs=2) as hxp,
            tc.tile_pool(name="sqp", bufs=2) as sqp,
            tc.tile_pool(name="op", bufs=2) as op,
            tc.tile_pool(name="statp", bufs=1) as statp,
            tc.tile_pool(name="ps_mm", bufs=4, space="PSUM") as ps_mm,
            tc.tile_pool(name="ps_acc", bufs=1, space="PSUM") as ps_acc,
            tc.tile_pool(name="ps_bc", bufs=1, space="PSUM") as ps_bc,
        ):
            b1s = constp.tile([P, NJ], f32)
            b2s = constp.tile([P, NK], f32)
            lgs = constp.tile([P, NJ], f32)
            lbs = constp.tile([P, NJ], f32)
            w1sh = constp.tile([P, DP, 2, 32], f8)
            w1sl = constp.tile([P, DP, 2, 32], f8)
            sb1h = constp.tile([1, 1], f32)

            def emit_const_dmas():
                nc.sync.dma_start(b1s[:], b1[:])
                nc.sync.dma_start(lgs[:], lg[:])
                nc.sync.dma_start(lbs[:], lb[:])
                nc.sync.dma_start(w1sh[:], W1SH[:])
                nc.sync.dma_start(w1sl[:], W1SL[:])
                nc.sync.dma_start(sb1h[:], SB1H[:])

            ones_q = constp.tile([P, 2, 32], f8)   # lhsT for Q paired DR sums
            nc.any.memset(ones_q[:], 1.0)
            oner_f = constp.tile([1, P], f32)
            nc.any.memset(oner_f[:], 1.0)
            oner_c = constp.tile([1, P], f32r)     # lhsT for partition-broadcasts
            nc.vector.tensor_copy(oner_c[:], oner_f[:])
            eps_t = constp.tile([1, 1], f32)
            nc.any.memset(eps_t[:], LN_EPS)

            # fp8 weights, SBUF-resident for the whole kernel, streamed in
            # contiguous per-chunk DMAs staged around the first two tiles.
            w1h = wp.tile([P, NJ, DP, 2, P], f8)
            w1l = wp.tile([P, NJ, DP, 2, P], f8)
            w2h = wp.tile([P, NK, KP, 2, P], f8)
            w2l = wp.tile([P, NK, KP, 2, P], f8)

            def emit_w1(a, b):
                nc.sync.dma_start(w1h[:, a:b], W1H[:, a:b])
                nc.sync.dma_start(w1l[:, a:b], W1L[:, a:b])

            def emit_w2(a, b):
                nc.sync.dma_start(w2h[:, a:b], W2H[:, a:b])
                nc.sync.dma_start(w2l[:, a:b], W2L[:, a:b])

            def emit_mm2(h_hi, h_lo, t0, tt, mid=None, post=None):
                # 3-pass compensated fp8 mm2 + b2-fold, evict on DVE.
                # mid() runs after chain 5 (the next tile's stats-broadcast
                # matmuls); post(j) runs twice per chain from chain 6 (the
                # next tile's normalize/GELU work).
                step = 0
                for k in range(NK):
                    pm = ps_mm.tile([P, TT], f32, tag="mm", name="mm2")[:, :tt]
                    for pi, (wt, ht) in enumerate(
                        ((w2h, h_hi), (w2h, h_lo), (w2l, h_hi))
                    ):
                        for kp in range(KP):
                            nc.tensor.matmul(
                                pm[:],
                                wt[:, k, kp, :, :],
                                ht[:, kp, :, :tt],
                                start=(pi == 0 and kp == 0),
                                stop=(pi == 2 and kp == KP - 1),
                                perf_mode=DR,
                            )
                    ot = op.tile([P, tt], bf16, tag=f"out{tt}", name="out", bufs=4)
                    nc.vector.tensor_scalar(
                        ot[:], pm[:], 1.0 / SW2, b2s[:, k : k + 1], Mul, Add
                    )
                    nc.sync.dma_start(outT[k, :, t0 : t0 + tt], ot[:])
                    if k == 5 and mid is not None:
                        mid()
                    if k >= 6 and post is not None:
                        for _ in range(2):
                            if step < NJ:
                                post(step)
                                step += 1
                while post is not None and step < NJ:
                    post(step)
                    step += 1

            prev = None
            backlog = []
            tiles = _t_tiles(C)
            for tile_i, (t0, tt) in enumerate(tiles):
                if tile_i == 0:
                    emit_w1(0, 1)  # W1 j=0 ahead of x so the first chain starts fast
                xh = xp.tile([P, DP, 2, TT], f8, tag="xh", name="xh")
                xl = xp.tile([P, DP, 2, TT], f8, tag="xl", name="xl")
                nc.sync.dma_start(xh[:, :, :, :tt], XH[:, :, :, t0 : t0 + tt])
                nc.sync.dma_start(xl[:, :, :, :tt], XL[:, :, :, t0 : t0 + tt])
                if tile_i == 0:
                    emit_w1(1, 3)
                    emit_const_dmas()
                h = hp.tile(
                    [P, NJ, tt], bf16, tag=f"h{tt}", name="h",
                    bufs=(2 if tt == TT else 1),
                )
                h_hi = hxp.tile(
                    [P, KP, 2, tt], f8, tag=f"hh{tt}", name="h_hi",
                    bufs=(2 if tt == TT else 1),
                )
                h_lo = hxp.tile(
                    [P, KP, 2, tt], f8, tag=f"hl{tt}", name="h_lo",
                    bufs=(2 if tt == TT else 1),
                )
                s_ps = ps_acc.tile([32, TT], f32, tag="sacc", name="sacc")[:, :tt]
                q_ps = ps_acc.tile([32, TT], f32, tag="qacc", name="qacc")[:, :tt]

                # ---- mm1 (3-pass fp8 DR); Q ones-matmuls deferred one pair
                # so the PE never waits on the ACT evict / DVE square chain;
                # tile0's norm/GELU backlog drip-fed through tile1's loop ----
                pend_q = None
                sq = None
                for j in range(NJ):
                    if tile_i == 0:
                        if j == 0:
                            emit_w1(3, 8)
                        elif j == 4:
                            emit_w1(8, NJ)
                        elif j == 8:
                            emit_w2(0, 4)
                        elif j == 12:
                            emit_w2(4, 8)
                    elif tile_i == 1:
                        if j == 0:
                            emit_w2(8, 12)
                        elif j == 4:
                            emit_w2(12, NK)
                        elif j == 8:
                            nc.sync.dma_start(b2s[:], b2[:])
                    pm = ps_mm.tile([P, TT], f32, tag="mm", name="mm1")[:, :tt]
                    for pi, (wt, xt) in enumerate(((w1h, xh), (w1h, xl), (w1l, xh))):
                        for dp in range(DP):
                            nc.tensor.matmul(
                                pm[:],
                                wt[:, j, dp, :, :],
                                xt[:, dp, :, :tt],
                                start=(pi == 0 and dp == 0),
                                stop=(pi == 2 and dp == DP - 1),
                                perf_mode=DR,
                            )
                    nc.scalar.activation(
                        h[:, j, :], pm[:], Ident,
                        bias=b1s[:, j : j + 1], scale=1.0 / (SX * SW1),
                    )
                    if j % 2 == 0:
                        sq = sqp.tile([P, 2, TT], f8, tag="sq", name="sq")
                    nc.vector.tensor_mul(sq[:, j % 2, :tt], h[:, j, :], h[:, j, :])
                    if j % 2 == 1:
                        if pend_q is not None:
                            jp, sqt = pend_q
                            nc.tensor.matmul(
                                q_ps[:], ones_q[:], sqt[:, :, :tt],
                                start=(jp == 0), stop=(jp == NJ // 2 - 1),
                                perf_mode=DR,
                            )
                        pend_q = (j // 2, sq)
                    if backlog:
                        backlog.pop(0)()
                while backlog:  # finish tile0's backlog before mm2(0) reads h
                    backlog.pop(0)()
                # S-fold: the W1-column-sum row, 3-pass DR into s_ps
                for pi, (wt, xt) in enumerate(((w1sh, xh), (w1sh, xl), (w1sl, xh))):
                    for dp in range(DP):
                        nc.tensor.matmul(
                            s_ps[:],
                            wt[:, dp, :, :],
                            xt[:, dp, :, :tt],
                            start=(pi == 0 and dp == 0),
                            stop=(pi == 2 and dp == DP - 1),
                            perf_mode=DR,
                        )
                jp, sqt = pend_q
                nc.tensor.matmul(
                    q_ps[:], ones_q[:], sqt[:, :, :tt],
                    start=(jp == 0), stop=(jp == NJ // 2 - 1),
                    perf_mode=DR,
                )

                # ---- LN stats (DVE/ACT only; broadcasts happen mid-mm2) ----
                mu = statp.tile([1, TT], f32, tag="mu", name="mu")[:, :tt]
                nc.vector.tensor_scalar(
                    mu[:], s_ps[0:1, :], 1.0 / (SX * SW1S * H), sb1h[:], Mul, Add
                )
                tmp = statp.tile([1, TT], f32, tag="tmp", name="tmp")[:, :tt]
                nc.vector.tensor_scalar_mul(tmp[:], q_ps[0:1, :], 1.0 / H)
                tmp2 = statp.tile([1, TT], f32, tag="tmp2", name="tmp2")[:, :tt]
                nc.vector.tensor_mul(tmp2[:], mu[:], mu[:])
                nc.vector.tensor_sub(tmp[:], tmp[:], tmp2[:])          # var
                nc.scalar.activation(tmp2[:], tmp[:], Sqrt, bias=eps_t[:])  # std
                nc.vector.reciprocal(tmp[:], tmp2[:])                  # rstd
                a_row = statp.tile([1, TT], f32r, tag="a_row", name="a_row")[:, :tt]
                nc.vector.tensor_copy(a_row[:], tmp[:])
                b_row = statp.tile([1, TT], f32r, tag="b_row", name="b_row")[:, :tt]
                nc.vector.tensor_mul(b_row[:], mu[:], tmp[:])

                a_sb = statp.tile([P, TT], bf16, tag="a_sb", name="a_sb", bufs=2)
                b_sb = statp.tile([P, TT], bf16, tag="b_sb", name="b_sb", bufs=2)

                def emit_bc(a_row=a_row, b_row=b_row, a_sb=a_sb, b_sb=b_sb, tt=tt):
                    a_bc = ps_bc.tile([P, TT], f32, tag="a_bc", name="a_bc")[:, :tt]
                    nc.tensor.matmul(
                        a_bc[:], oner_c[:], a_row[:], start=True, stop=True
                    )
                    b_bc = ps_bc.tile([P, TT], f32, tag="b_bc", name="b_bc")[:, :tt]
                    nc.tensor.matmul(
                        b_bc[:], oner_c[:], b_row[:], start=True, stop=True
                    )
                    nc.vector.tensor_copy(a_sb[:, :tt], a_bc[:])
                    nc.vector.tensor_copy(b_sb[:, :tt], b_bc[:])

                def emit_norm_gelu(
                    j, h=h, h_hi=h_hi, h_lo=h_lo, a_sb=a_sb, b_sb=b_sb, tt=tt
                ):
                    # normalize (DVE bf16) + GELU twice (ACT: fp8 h_hi, bf16
                    # in place) + h_lo residual (DVE)
                    jp, pl = j // 2, j % 2
                    hj = h[:, j, :tt]
                    nc.vector.tensor_mul(hj, hj, a_sb[:, :tt])
                    nc.vector.tensor_sub(hj, hj, b_sb[:, :tt])
                    nc.scalar.activation(
                        h_hi[:, jp, pl, :tt], hj, Gelu,
                        bias=lbs[:, j : j + 1], scale=lgs[:, j : j + 1],
                    )
                    nc.scalar.activation(
                        hj, hj, Gelu, bias=lbs[:, j : j + 1], scale=lgs[:, j : j + 1]
                    )
                    nc.vector.tensor_sub(
                        h_lo[:, jp, pl, :tt], hj, h_hi[:, jp, pl, :tt]
                    )

                # ---- previous tile's mm2 on the PE, with this tile's
                # broadcasts at chain 5 and norm/GELU from chain 6 ----
                if prev is not None:
                    emit_mm2(*prev, mid=emit_bc, post=emit_norm_gelu)
                else:
                    backlog.append(emit_bc)
                    backlog.extend(
                        (lambda j=j, f=emit_norm_gelu: f(j)) for j in range(NJ)
                    )
                prev = (h_hi, h_lo, t0, tt)

            if len(tiles) == 1:  # safety for tiny C: no tile-1 DMA slots
                emit_w2(8, NK)
                nc.sync.dma_start(b2s[:], b2[:])
            for fn in backlog:
                fn()
            emit_mm2(*prev)

    nc.compile()
    return nc


def _route(x64, Wg64, bg64):
    """Host gating: per-token top-2 expert ids and renormalized weights."""
    logits = x64 @ Wg64 + bg64                      # [N, E] fp64
    order = np.argsort(-logits, axis=1, kind="stable")[:, :TOPK]
    l0 = np.take_along_axis(logits, order, axis=1)  # [N, 2] descending
    w0 = 1.0 / (1.0 + np.exp(l0[:, 1] - l0[:, 0]))
    w = np.stack([w0, 1.0 - w0], axis=1)
    return order, w


def _split8(a):
    hi = a.astype(F8)
    lo = (a - hi.astype(np.float32)).astype(F8)
    return hi, lo


def kernel(x, W1, b1, ln_g, ln_b, W2, b2, Wg, bg):
    x = np.ascontiguousarray(np.asarray(x, dtype=np.float32))
    W1 = np.asarray(W1, dtype=np.float32)
    b1 = np.asarray(b1, dtype=np.float32)
    ln_g = np.asarray(ln_g, dtype=np.float32)
    ln_b = np.asarray(ln_b, dtype=np.float32)
    W2 = np.asarray(W2, dtype=np.float32)
    b2 = np.asarray(b2, dtype=np.float32)
    Wg = np.asarray(Wg, dtype=np.float32)
    bg = np.asarray(bg, dtype=np.float32)
    N = x.shape[0]

    order, w = _route(x.astype(np.float64), Wg.astype(np.float64), bg.astype(np.float64))

    tok_idx, tok_w = [], []
    for e in range(E):
        sel = np.nonzero((order[:, 0] == e) | (order[:, 1] == e))[0]
        we = np.where(order[sel, 0] == e, w[sel, 0], w[sel, 1]).astype(np.float32)
        tok_idx.append(sel)
        tok_w.append(we)
    C = max(GRAN, int(-(-max(len(s) for s in tok_idx) // GRAN)) * GRAN)

    if C not in _kernel_cache:
        _kernel_cache[C] = _build(C)
    nc = _kernel_cache[C]

    in_maps = []
    for e in range(E):
        idx = np.zeros(C, dtype=np.int64)
        idx[: len(tok_idx[e])] = tok_idx[e]
        xg = x[idx] * SX                              # [C, D]
        xh, xl = _split8(xg)
        # [C, D] -> [P, DP, 2, C]
        xh_d = np.ascontiguousarray(xh.reshape(C, DP, 2, P).transpose(3, 1, 2, 0))
        xl_d = np.ascontiguousarray(xl.reshape(C, DP, 2, P).transpose(3, 1, 2, 0))
        w1h, w1l = _split8(W1[e] * SW1)               # [D, H]
        w1h_d = np.ascontiguousarray(
            w1h.reshape(DP, 2, P, NJ, P).transpose(2, 3, 0, 1, 4)
        )
        w1l_d = np.ascontiguousarray(
            w1l.reshape(DP, 2, P, NJ, P).transpose(2, 3, 0, 1, 4)
        )
        # S-fold: column-sum of W1 (scaled), replicated over 32 lhsT columns
        w1s = W1[e].sum(axis=1) * SW1S                # [D]
        w1sh, w1sl = _split8(w1s)
        w1sh_d = np.ascontiguousarray(np.broadcast_to(
            w1sh.reshape(DP, 2, P).transpose(2, 0, 1)[:, :, :, None], (P, DP, 2, 32)
        ).astype(F8))
        w1sl_d = np.ascontiguousarray(np.broadcast_to(
            w1sl.reshape(DP, 2, P).transpose(2, 0, 1)[:, :, :, None], (P, DP, 2, 32)
        ).astype(F8))
        sb1h_d = np.full((1, 1), b1[e].sum() / H, dtype=np.float32)
        w2h, w2l = _split8(W2[e] * SW2)               # [H, H]
        w2h_d = np.ascontiguousarray(
            w2h.reshape(KP, 2, P, NK, P).transpose(2, 3, 0, 1, 4)
        )
        w2l_d = np.ascontiguousarray(
            w2l.reshape(KP, 2, P, NK, P).transpose(2, 3, 0, 1, 4)
        )
        in_maps.append(
            {
                "XH": xh_d,
                "XL": xl_d,
                "W1H": w1h_d,
                "W1L": w1l_d,
                "W1SH": w1sh_d,
                "W1SL": w1sl_d,
                "SB1H": sb1h_d,
                "W2H": w2h_d,
                "W2L": w2l_d,
                "b1": np.ascontiguousarray(b1[e].reshape(NJ, P).T),
                "b2": np.ascontiguousarray(b2[e].reshape(NK, P).T),
                "lg": np.ascontiguousarray(ln_g[e].reshape(NJ, P).T),
                "lb": np.ascontiguousarray(ln_b[e].reshape(NJ, P).T),
            }
        )

    results = _run(C, nc, in_maps)

    y = np.zeros((N, H), dtype=np.float32)
    for e in range(E):
        cnt = len(tok_idx[e])
        eoT = results[e]["outT"].reshape(H, C).astype(np.float32)
        y[tok_idx[e]] += tok_w[e][:, None] * eoT[:, :cnt].T
    return y


_neff_cache: dict[int, str] = {}


def _run(C, nc, in_maps):
    if axon_active():
        # PJRT path; NEFF compile is cached by libneuronxla.
        return run_bass_kernel_spmd(nc, in_maps, core_ids=list(range(E))).results
    # Native path: compile once per capacity, then execute the cached NEFF.
    from concourse.bass_utils import compile_bass_kernel, run_neff

    if C not in _neff_cache:
        _neff_cache[C] = compile_bass_kernel(nc, tempfile.mkdtemp())
    out_maps = [{"outT": np.zeros((NK, P, C), dtype=BF)} for _ in range(E)]
    in_maps = [m.copy() for m in in_maps]
    if nc.partition_id_tensor:
        for core_id, m in enumerate(in_maps):
            m[nc.partition_id_tensor.name] = np.array([[core_id]], dtype=np.uint32)
    return run_neff(
        _neff_cache[C],
        in_maps,
        out_maps,
        core_ids=list(range(E)),
        has_collectives=False,
    )
